# revision 1
# baseline (speedup 1.0000x reference)
"""Trainium2 Bass kernel for GQA attention (B=2, S=2048, DIM=2048, H=32, KV=8, HD=64).

Sharding: tensor-parallel over kv heads (TP=4, 2 kv heads / 8 q heads per core)
x data-parallel over batch (DP=2).  Core c = d*4 + t.  Each core computes a
partial out = attn_out_shard @ wo_rows_shard for its batch; the host sums the
4 TP partials per batch.

All host-side work is layout-only: transpose x, permute wq/wk columns into a
RoPE-friendly even/odd layout, cast to bf16, build trig/mask pattern tiles.

Device dataflow (per core):
 - projections with x^T resident in SBUF (bf16 matmuls, k-outer accumulation)
 - RoPE via stream_shuffle + two tensor muls + add (even/odd pairs laid out
   within 32-partition quadrants)
 - attention with transposed scores (scores[sk, sq]) so no transposes are
   needed anywhere in the inner loop; exp on ScalarE with no max-subtraction
   (inputs are unit-scale; softmax is shift-invariant)
 - causal masking by block skipping + multiplicative 0/1 patterns on the
   diagonal tiles, with column-trimmed exp/mask/av on those tiles
 - softmax denominators ride as a ones-column inside the AV matmul lhsT;
   normalization = DVE reciprocal -> gpsimd partition_broadcast -> DVE mul
 - wo output projection as a final phase, outputs streamed to DRAM
"""

import os
import sys

import numpy as np

_REPO = "/opt/trn_rl_repo"
if _REPO not in sys.path:
    sys.path.insert(0, _REPO)

import ml_dtypes  # noqa: E402

BF16 = ml_dtypes.bfloat16

B, S, DIM = 2, 2048, 2048
H, KV, HD = 32, 8, 64
TP, DP = 4, 2
NCORES = TP * DP
HQ = (H // TP) * HD          # 512 q-proj cols per core
HKV = (KV // TP) * HD        # 128 kv-proj cols per core
NKVC = KV // TP              # 2 kv heads per core
NPAIR = (H // TP) // 2       # 4 q-head pairs per core
SQC = 512                    # sq chunk width
NCHUNK = S // SQC
SKT = 128                    # sk tile height
NSKT = S // SKT
KT = DIM // 128              # contraction tiles
VW = 130                     # v_sb tile: [0(32) | 1 | 0(31) | v(64) | 1 | pad]

# RoPE layout: within each head's 64 dims -> 64 partitions, quadrant q (32)
# holds pairs 16q..16q+15 as [evens(16) | odds(16)].
_perm = np.empty(64, np.int64)
_freq = np.empty(64, np.int64)
_sgn = np.empty(64, np.float32)
for _p in range(64):
    _q, _j = divmod(_p, 32)
    if _j < 16:
        _i = 16 * _q + _j
        _perm[_p] = 2 * _i
        _sgn[_p] = -1.0
    else:
        _i = 16 * _q + _j - 16
        _perm[_p] = 2 * _i + 1
        _sgn[_p] = 1.0
    _freq[_p] = _i
SHUF = list(range(16, 32)) + list(range(0, 16))

_build_cache = {}
last_exec_time_ns = None
last_trace = None


def _mask_structure(mask):
    """chunks[c] = [(t, pat_idx|None, col_trim), ...] per valid sk tile;
    patterns = list of [128, 2*SQC] float32 0/1 (duplicated for both halves
    of the mixed-half p tile)."""
    valid = mask[0, 0] == 0.0  # [sq, sk]
    chunks = []
    patterns = []
    pat_keys = {}
    for c in range(NCHUNK):
        glist = []
        for t in range(NSKT):
            sub = valid[c * SQC:(c + 1) * SQC, t * SKT:(t + 1) * SKT]
            if not sub.any():
                continue
            if sub.all():
                glist.append((t, None, 0))
                continue
            pat = np.empty((128, 2 * SQC), np.float32)
            pat[:, 0:SQC] = (sub.T - 1.0) * 240.0
            pat[:, SQC:2 * SQC] = pat[:, 0:SQC]
            key = pat.tobytes()
            if key not in pat_keys:
                pat_keys[key] = len(patterns)
                patterns.append(pat)
            # first sq column with any valid element: exp/mask/av can skip
            # columns < r (their p values are zero / never contribute)
            r = int(np.argmax(sub.any(axis=1)))
            glist.append((t, pat_keys[key], r))
        chunks.append(glist)
    return chunks, patterns


def _build(chunks, n_pat):
    import concourse.bass as bass  # noqa: F401
    import concourse.mybir as mybir
    from concourse import bacc
    from concourse.masks import make_identity
    from concourse.tile import TileContext

    F32, BF = mybir.dt.float32, mybir.dt.bfloat16
    MUL = mybir.AluOpType.mult
    ADD = mybir.AluOpType.add
    EXP = mybir.ActivationFunctionType.Exp

    nc = bacc.Bacc()
    xt_e = nc.declare_dram_parameter("xt", [DIM, S], BF, isOutput=False)
    wq_e = nc.declare_dram_parameter("wq", [DIM, HQ], BF, isOutput=False)
    wk_e = nc.declare_dram_parameter("wk", [DIM, HKV], BF, isOutput=False)
    wv_e = nc.declare_dram_parameter("wv", [DIM, HKV], BF, isOutput=False)
    wo_e = nc.declare_dram_parameter("wo", [HQ, DIM], BF, isOutput=False)
    c1_e = nc.declare_dram_parameter("c1", [128, S], BF, isOutput=False)
    c2_e = nc.declare_dram_parameter("c2", [128, S], BF, isOutput=False)
    dm_e = nc.declare_dram_parameter("dmask", [128, n_pat * 2 * SQC], BF,
                                     isOutput=False)
    out_e = nc.declare_dram_parameter("out", [S, DIM], BF, isOutput=True)

    with TileContext(nc) as tc:
        with tc.tile_pool(name="persist", bufs=1) as P:
            q_t = [P.tile([128, S], BF, tag=f"q{j}", name=f"q{j}")
                   for j in range(NPAIR)]
            k_t = P.tile([128, S], BF, tag="kt")
            v_sb = [P.tile([128, NSKT * VW], BF, tag=f"v{g}", name=f"v{g}")
                    for g in range(NKVC)]
            attn = [P.tile([128, S], BF, tag=f"a{j}", name=f"a{j}")
                    for j in range(NPAIR)]
            wo_sb = [P.tile([128, DIM], BF, tag=f"wo{j}", name=f"wo{j}")
                     for j in range(NPAIR)]
            dm_sb = P.tile([128, n_pat * 2 * SQC], BF, tag="dm")
            ident = P.tile([128, 128], BF, tag="ident")
            make_identity(nc, ident)

            for j in range(NPAIR):
                nc.gpsimd.dma_start(out=wo_sb[j],
                                    in_=wo_e[128 * j:128 * (j + 1), :])
            nc.gpsimd.dma_start(out=dm_sb, in_=dm_e[:, :])

            # v background: [0(32) | 1 | 0(31) | v | 1 | pad] per sk tile
            for g in range(NKVC):
                v3 = v_sb[g].rearrange("p (t w) -> p t w", w=VW)
                nc.vector.memset(v3[:, :, 0:32], 0.0)
                nc.vector.memset(v3[:, :, 32:33], 1.0)
                nc.vector.memset(v3[:, :, 33:64], 0.0)
                nc.vector.memset(v3[:, :, 128:129], 1.0)

            # ---------------- projections ----------------
            with (
                tc.tile_pool(name="xw", bufs=1) as XW,
                tc.tile_pool(name="ropew", bufs=2) as W,
                tc.tile_pool(name="pps", bufs=2, space="PSUM") as PPS,
            ):
                c1_sb = XW.tile([128, S], BF, tag="c1")
                c2_sb = XW.tile([128, S], BF, tag="c2")
                nc.gpsimd.dma_start(out=c1_sb, in_=c1_e[:, :])
                nc.gpsimd.dma_start(out=c2_sb, in_=c2_e[:, :])
                # x on the sync HWDGE queue, weights on the scalar queue, so
                # weight tiles land in parallel with the big x stream
                xt_sb, wq_sb, wk_sb, wv_sb = [], [], [], []
                for k in range(KT):
                    xq = nc.sync if k % 2 == 0 else nc.scalar
                    wqq = nc.scalar if k % 2 == 0 else nc.sync
                    xk = XW.tile([128, S], BF, tag=f"x{k}")
                    xq.dma_start(out=xk,
                                 in_=xt_e[128 * k:128 * (k + 1), :])
                    xt_sb.append(xk)
                    kk = XW.tile([128, HKV], BF, tag=f"wk{k}")
                    wqq.dma_start(out=kk,
                                  in_=wk_e[128 * k:128 * (k + 1), :])
                    wk_sb.append(kk)
                    vk = XW.tile([128, HKV], BF, tag=f"wv{k}")
                    wqq.dma_start(out=vk,
                                  in_=wv_e[128 * k:128 * (k + 1), :])
                    wv_sb.append(vk)
                    qk_ = XW.tile([128, HQ], BF, tag=f"wq{k}")
                    wqq.dma_start(out=qk_,
                                  in_=wq_e[128 * k:128 * (k + 1), :])
                    wq_sb.append(qk_)

                def rope(dst, raw):
                    # dst = raw*c1 + shuffle(raw)*c2
                    sh = W.tile([128, S], BF, tag="sh", name="sh")
                    t1 = W.tile([128, S], BF, tag="t1", name="t1")
                    nc.vector.stream_shuffle(sh, raw, SHUF)
                    nc.vector.tensor_tensor(t1, raw, c1_sb, MUL)
                    nc.vector.tensor_tensor(sh, sh, c2_sb, MUL)
                    nc.vector.tensor_tensor(dst, t1, sh, ADD)

                def rope_project(dst, w_tiles, col0):
                    # k-outer: one ldweights feeds 4 chunk matmuls
                    raw = W.tile([128, S], BF, tag="qraw", name="raw")
                    _tags = ["ppk0", "ppk1", "ppv0", "ppv1"]
                    pss = [PPS.tile([128, SQC], F32, tag=_tags[c],
                                    name=f"pq{c}", bufs=1)
                           for c in range(NCHUNK)]
                    for k in range(KT):
                        for c in range(NCHUNK):
                            nc.tensor.matmul(
                                pss[c],
                                w_tiles[k][:, col0:col0 + 128],
                                xt_sb[k][:, SQC * c:SQC * (c + 1)],
                                start=(k == 0), stop=(k == KT - 1),
                            )
                    for c in range(NCHUNK):
                        nc.scalar.copy(raw[:, SQC * c:SQC * (c + 1)], pss[c])
                    rope(dst, raw)

                # k / v_t / q0 interleaved per k-tile in two chunk passes so
                # the PE has dense work while x/w tiles stream in
                raw_k = W.tile([128, S], BF, tag="rawk", bufs=1)
                raw_q0 = W.tile([128, S], BF, tag="rawq0", bufs=1)
                raw_q1 = W.tile([128, S], BF, tag="rawq1", bufs=1)
                vt_raw = W.tile([128, S], BF, tag="rawv", bufs=1)
                for crng in ((0, 1), (2, 3)):
                    ps_k = [PPS.tile([128, SQC], F32, tag=f"ppk{i}",
                                     name=f"ppk{i}", bufs=1)
                            for i in range(2)]
                    ps_v = [PPS.tile([128, SQC], F32, tag=f"ppv{i}",
                                     name=f"ppv{i}", bufs=1)
                            for i in range(2)]
                    ps_q = [PPS.tile([128, SQC], F32, tag=f"ppq{i}",
                                     name=f"ppq{i}", bufs=1)
                            for i in range(2)]
                    ps_q1 = [PPS.tile([128, SQC], F32, tag=f"ppr{i}",
                                      name=f"ppr{i}", bufs=1)
                             for i in range(2)]
                    for k in range(KT):
                        for ci, c in enumerate(crng):
                            xs = xt_sb[k][:, SQC * c:SQC * (c + 1)]
                            nc.tensor.matmul(
                                ps_k[ci], wk_sb[k], xs,
                                start=(k == 0), stop=(k == KT - 1))
                            nc.tensor.matmul(
                                ps_v[ci], wv_sb[k], xs,
                                start=(k == 0), stop=(k == KT - 1))
                            nc.tensor.matmul(
                                ps_q[ci], wq_sb[k][:, 0:128], xs,
                                start=(k == 0), stop=(k == KT - 1))
                            nc.tensor.matmul(
                                ps_q1[ci], wq_sb[k][:, 128:256], xs,
                                start=(k == 0), stop=(k == KT - 1))
                    for ci, c in enumerate(crng):
                        sl = slice(SQC * c, SQC * (c + 1))
                        nc.scalar.copy(raw_k[:, sl], ps_k[ci])
                        nc.scalar.copy(vt_raw[:, sl], ps_v[ci])
                        nc.scalar.copy(raw_q0[:, sl], ps_q[ci])
                        nc.scalar.copy(raw_q1[:, sl], ps_q1[ci])
                rope(k_t, raw_k)
                rope(q_t[0], raw_q0)
                rope(q_t[1], raw_q1)
                for t in range(NSKT):
                    tp = PPS.tile([128, 128], BF, tag="ppk0", bufs=1)
                    nc.tensor.transpose(tp, vt_raw[:, SKT * t:SKT * (t + 1)],
                                        ident)
                    nc.scalar.copy(
                        v_sb[0][:, VW * t + 64:VW * t + 128], tp[:, 0:64])
                    nc.scalar.copy(
                        v_sb[1][:, VW * t + 64:VW * t + 128], tp[:, 64:128])

                for j in range(2, NPAIR):
                    rope_project(q_t[j], wq_sb, 128 * j)

            # ---------------- attention ----------------
            with (
                tc.tile_pool(name="attw", bufs=2) as W,
                tc.tile_pool(name="scps", bufs=2, space="PSUM") as SCPS,
                tc.tile_pool(name="avps", bufs=1, space="PSUM") as AVPS,
            ):
                for c in range(NCHUNK):
                    glist = chunks[c]
                    for j in range(NPAIR):
                        # pair j = (q-head j -> kv 0, q-head j+4 -> kv 1)
                        # mixed-half sc tile: lo head at cols 0:512 (PE rows
                        # 0-63), hi head at cols 512:1024 (rows 64-127) --
                        # the two qk matmuls run concurrently in the array
                        pp = j % 2
                        av_lo = AVPS.tile([128, SQC], F32, tag=f"avlo{pp}",
                                          name="av_lo")
                        av_hi = AVPS.tile([128, SQC], F32, tag=f"avhi{pp}",
                                          name="av_hi")
                        for ti, (t, patk, r) in enumerate(glist):
                            first = ti == 0
                            last = ti == len(glist) - 1
                            if first:
                                r = 0  # first av matmul must cover all cols
                            sc = SCPS.tile([128, 2 * SQC], F32,
                                           tag="sc", name="sc")
                            masked = patk is not None
                            nc.tensor.matmul(
                                sc[:, r:SQC],
                                k_t[0:64, SKT * t:SKT * (t + 1)],
                                q_t[j][0:64, SQC * c + r:SQC * (c + 1)],
                                start=True, stop=not masked,
                            )
                            nc.tensor.matmul(
                                sc[:, SQC + r:2 * SQC],
                                k_t[64:128, SKT * t:SKT * (t + 1)],
                                q_t[j][64:128, SQC * c + r:SQC * (c + 1)],
                                start=True, stop=not masked,
                            )
                            if masked:
                                # additive -240 mask via PE: sc += I.T @ pat
                                base = 2 * SQC * patk
                                nc.tensor.matmul(
                                    sc[:, r:SQC], ident,
                                    dm_sb[:, base + r:base + SQC],
                                    start=False, stop=True,
                                )
                                nc.tensor.matmul(
                                    sc[:, SQC + r:2 * SQC], ident,
                                    dm_sb[:, base + SQC + r:base + 2 * SQC],
                                    start=False, stop=True,
                                )
                            p = W.tile([128, 2 * SQC], BF, tag="p", name="p",
                                       bufs=4)
                            if r:
                                sc3 = sc.rearrange(
                                    "q (h f) -> q h f", h=2)[:, :, r:SQC]
                                p3 = p.rearrange(
                                    "q (h f) -> q h f", h=2)[:, :, r:SQC]
                                nc.scalar.activation(p3, sc3, EXP,
                                                     scale=0.125)
                            else:
                                nc.scalar.activation(p, sc, EXP, scale=0.125)
                            nc.tensor.matmul(
                                av_lo[0:65, r:SQC],
                                v_sb[0][:, VW * t + 64:VW * t + 129],
                                p[:, r:SQC],
                                start=first, stop=last,
                            )
                            nc.tensor.matmul(
                                av_hi[0:128, r:SQC],
                                v_sb[1][:, VW * t:VW * t + 128],
                                p[:, SQC + r:2 * SQC],
                                start=first, stop=last,
                            )
                        rec_lo = W.tile([1, SQC], F32, tag="reclo",
                                        name="rec_lo")
                        rec_hi = W.tile([1, SQC], F32, tag="rechi",
                                        name="rec_hi")
                        rb_lo = W.tile([128, SQC], F32, tag="rblo",
                                       name="rb_lo")
                        rb_hi = W.tile([128, SQC], F32, tag="rbhi",
                                       name="rb_hi")
                        nc.vector.reciprocal(rec_lo[0:1, :], av_lo[64:65, :])
                        nc.gpsimd.partition_broadcast(rb_lo, rec_lo[0:1, :])
                        nc.vector.tensor_tensor(
                            attn[j][0:64, SQC * c:SQC * (c + 1)],
                            av_lo[0:64, :], rb_lo[0:64, :], MUL)
                        nc.vector.reciprocal(rec_hi[0:1, :], av_hi[32:33, :])
                        nc.gpsimd.partition_broadcast(rb_hi, rec_hi[0:1, :])
                        nc.vector.tensor_tensor(
                            attn[j][64:128, SQC * c:SQC * (c + 1)],
                            av_hi[64:128, :], rb_hi[64:128, :], MUL)

            # ---------------- wo (output projection) ----------------
            with (
                tc.tile_pool(name="wow", bufs=3) as W2,
                tc.tile_pool(name="ops", bufs=4, space="PSUM") as OPS,
            ):
                for s in range(S // 128):
                    o_sb = W2.tile([128, DIM], BF, tag="osb", name="o_sb")
                    for n in range(DIM // 512):
                        pso = OPS.tile([128, 512], F32, tag="pso",
                                       name="pso")
                        for j in range(NPAIR):
                            nc.tensor.matmul(
                                pso,
                                attn[j][:, 128 * s:128 * (s + 1)],
                                wo_sb[j][:, 512 * n:512 * (n + 1)],
                                start=(j == 0), stop=(j == NPAIR - 1),
                            )
                        dst = o_sb[:, 512 * n:512 * (n + 1)]
                        if n % 2 == 0:
                            nc.vector.tensor_copy(dst, pso)
                        else:
                            nc.scalar.copy(dst, pso)
                    nc.sync.dma_start(out=out_e[128 * s:128 * (s + 1), :],
                                      in_=o_sb)

    nc.finalize()
    return nc


def kernel(**inputs):
    global last_exec_time_ns, last_trace
    from concourse.bass_utils import run_bass_kernel_spmd

    x = np.asarray(inputs["x"], np.float32)
    freqs_cos = np.asarray(inputs["freqs_cos"], np.float32)
    freqs_sin = np.asarray(inputs["freqs_sin"], np.float32)
    mask = np.asarray(inputs["mask"], np.float32)
    wq = np.asarray(inputs["wq"], np.float32)
    wk = np.asarray(inputs["wk"], np.float32)
    wv = np.asarray(inputs["wv"], np.float32)
    wo = np.asarray(inputs["wo"], np.float32)

    chunks, patterns = _mask_structure(mask)
    n_pat = max(len(patterns), 1)
    if patterns:
        dmask = np.concatenate(patterns, axis=1).astype(BF16)
    else:
        dmask = np.ones((128, 2 * SQC), np.float32).astype(BF16)

    key = tuple(tuple(g) for g in chunks)
    if key not in _build_cache:
        _build_cache[key] = _build(chunks, n_pat)
    nc = _build_cache[key]

    # trig tiles in pair layout (same for both heads of a pair)
    fi2 = np.tile(_freq, 2)
    sg2 = np.tile(_sgn, 2)
    c1 = freqs_cos.T[fi2].astype(BF16)                      # [128, S]
    c2 = (freqs_sin.T[fi2] * sg2[:, None]).astype(BF16)     # [128, S]

    # pair j holds (q-head j, q-head j+4) so lo half uses kv 0, hi half kv 1
    pair_order = [0, 4, 1, 5, 2, 6, 3, 7]
    q_cols = np.concatenate([64 * pair_order[i] + _perm
                             for i in range(H // TP)])
    o_rows = np.concatenate([np.arange(64 * pair_order[i],
                                       64 * pair_order[i] + 64)
                             for i in range(H // TP)])
    kv_perm = np.concatenate([64 * h + _perm for h in range(KV // TP)])

    in_maps = []
    for d in range(DP):
        xt = np.ascontiguousarray(x[d].T).astype(BF16)
        for t in range(TP):
            wq_s = np.ascontiguousarray(
                wq[:, HQ * t:HQ * (t + 1)][:, q_cols]).astype(BF16)
            wk_s = np.ascontiguousarray(
                wk[:, HKV * t:HKV * (t + 1)][:, kv_perm]).astype(BF16)
            wv_s = np.ascontiguousarray(
                wv[:, HKV * t:HKV * (t + 1)]).astype(BF16)
            wo_s = np.ascontiguousarray(
                wo[HQ * t:HQ * (t + 1), :][o_rows]).astype(BF16)
            in_maps.append({
                "xt": xt, "wq": wq_s, "wk": wk_s, "wv": wv_s, "wo": wo_s,
                "c1": c1, "c2": c2, "dmask": dmask,
            })

    trace = bool(os.environ.get("BASS_KERNEL_TRACE"))
    res = run_bass_kernel_spmd(nc, in_maps, core_ids=list(range(NCORES)),
                               trace=trace)
    last_exec_time_ns = res.exec_time_ns
    last_trace = res
    out = np.empty((B, S, DIM), np.float32)
    for d in range(DP):
        acc = res.results[d * TP]["out"].astype(np.float32)
        for t in range(1, TP):
            acc = acc + res.results[d * TP + t]["out"]
        out[d] = acc
    return out



# revision 17
# speedup vs baseline: 1.0879x; 1.0879x over previous
"""Trainium2 Bass kernel for GQA attention (B=2, S=2048, DIM=2048, H=32, KV=8, HD=64).

Sharding: tensor-parallel over kv heads (TP=4, 2 kv heads / 8 q heads per core)
x data-parallel over batch (DP=2).  Core c = d*4 + t.  Each core computes a
partial out = attn_out_shard @ wo_rows_shard for its batch; the host sums the
4 TP partials per batch.

All host-side work is layout-only: transpose x, permute wq/wk columns into a
RoPE-friendly even/odd layout, cast to bf16, build trig/mask pattern tiles.

Device dataflow (per core):
 - projections with x^T resident in SBUF (bf16 matmuls, k-outer accumulation)
 - RoPE via stream_shuffle + two tensor muls + add
 - attention with transposed scores (scores[sk, sq]); exp on ScalarE
 - causal masking by block skipping + a single additive -240 lower-triangle
   [128,128] pattern applied (via PE) only to the diagonal 128-block of the
   four partial tiles per chunk, with column-trimmed exp/av on those tiles
 - softmax denominators ride as ones-columns in the AV lhsT at per-pair
   distinct output partitions (lo: 64+j, hi: 60+j), staged into one SBUF
   tile per chunk so a single DVE reciprocal covers 4 rows at once;
   normalization = gpsimd partition_broadcast -> DVE mul
 - wo output projection matmuls for chunk c-1 interleaved into chunk c's
   attention tiles as PE filler work; outputs streamed to DRAM
"""

import os
import sys

import numpy as np

_REPO = "/opt/trn_rl_repo"
if _REPO not in sys.path:
    sys.path.insert(0, _REPO)

import ml_dtypes  # noqa: E402

BF16 = ml_dtypes.bfloat16

B, S, DIM = 2, 2048, 2048
H, KV, HD = 32, 8, 64
TP, DP = 4, 2
NCORES = TP * DP
HQ = (H // TP) * HD          # 512 q-proj cols per core
HKV = (KV // TP) * HD        # 128 kv-proj cols per core
NKVC = KV // TP              # 2 kv heads per core
NPAIR = (H // TP) // 2       # 4 q-head pairs per core
SQC = 512                    # sq chunk width
NCHUNK = S // SQC
SKT = 128                    # sk tile height
NSKT = S // SKT
KT = DIM // 128              # contraction tiles
V0W = 96                     # v0 tile: [v(64) | ones(32)] -> denom row 64
V1W = 128                    # v1 tile: [0(32) | ones(32) | v(64)] -> denom row 32

# RoPE layout: within each head's 64 dims -> 64 partitions, quadrant q (32)
# holds pairs 16q..16q+15 as [evens(16) | odds(16)].
_perm = np.empty(64, np.int64)
_freq = np.empty(64, np.int64)
_sgn = np.empty(64, np.float32)
for _p in range(64):
    _q, _j = divmod(_p, 32)
    if _j < 16:
        _i = 16 * _q + _j
        _perm[_p] = 2 * _i
        _sgn[_p] = -1.0
    else:
        _i = 16 * _q + _j - 16
        _perm[_p] = 2 * _i + 1
        _sgn[_p] = 1.0
    _freq[_p] = _i
SHUF = list(range(16, 32)) + list(range(0, 16))

_build_cache = {}
last_exec_time_ns = None
last_trace = None


def _mask_structure(mask):
    """chunks[c] = [(t, diag, r), ...] per valid sk tile.  diag tiles get the
    fixed lower-triangle -240 pattern added to cols [r, r+128)."""
    valid = mask[0, 0] == 0.0  # [sq, sk]
    chunks = []
    for c in range(NCHUNK):
        glist = []
        for t in range(NSKT):
            sub = valid[c * SQC:(c + 1) * SQC, t * SKT:(t + 1) * SKT]
            if not sub.any():
                continue
            if sub.all():
                glist.append((t, False, 0))
                continue
            r = int(np.argmax(sub.any(axis=1)))
            # check the partial tile is the standard causal diagonal block:
            # valid iff sq >= r + sk_within_tile
            qq, kk = np.meshgrid(np.arange(SQC), np.arange(SKT), indexing="ij")
            assert (sub == (qq >= r + kk)).all(), "non-causal partial tile"
            glist.append((t, True, r))
        # full tiles first so the first av matmul covers all columns
        glist.sort(key=lambda g: g[1])
        chunks.append(tuple(glist))
    return tuple(chunks)


def _build(chunks):
    import concourse.bass as bass  # noqa: F401
    import concourse.mybir as mybir
    from concourse import bacc
    from concourse.masks import make_identity
    from concourse.tile import TileContext

    F32, BF = mybir.dt.float32, mybir.dt.bfloat16
    MUL = mybir.AluOpType.mult
    ADD = mybir.AluOpType.add
    EXP = mybir.ActivationFunctionType.Exp

    nc = bacc.Bacc()
    xt_e = nc.declare_dram_parameter("xt", [DIM, S], BF, isOutput=False)
    wq_e = nc.declare_dram_parameter("wq", [DIM, HQ], BF, isOutput=False)
    wk_e = nc.declare_dram_parameter("wk", [DIM, HKV], BF, isOutput=False)
    wv_e = nc.declare_dram_parameter("wv", [DIM, HKV], BF, isOutput=False)
    wo_e = nc.declare_dram_parameter("wo", [HQ, DIM], BF, isOutput=False)
    c1_e = nc.declare_dram_parameter("c1", [128, S], BF, isOutput=False)
    c2_e = nc.declare_dram_parameter("c2", [128, S], BF, isOutput=False)
    dm_e = nc.declare_dram_parameter("dmask", [128, 128], BF, isOutput=False)
    out_e = nc.declare_dram_parameter("out", [S, DIM], BF, isOutput=True)

    with TileContext(nc) as tc:
        with tc.tile_pool(name="persist", bufs=1) as P:
            q_t = [P.tile([128, S], BF, tag=f"q{j}", name=f"q{j}")
                   for j in range(NPAIR)]
            k_t = P.tile([128, S], BF, tag="kt")
            v0_sb = P.tile([128, NSKT * V0W], BF, tag="v0")
            v1_sb = P.tile([128, NSKT * V1W], BF, tag="v1")
            attn = [P.tile([128, S], BF, tag=f"a{j}", name=f"a{j}")
                    for j in range(NPAIR)]
            wo_sb = [P.tile([128, DIM], BF, tag=f"wo{j}", name=f"wo{j}")
                     for j in range(NPAIR)]
            dm_sb = P.tile([128, 128], BF, tag="dm")
            ident = P.tile([128, 128], BF, tag="ident")
            make_identity(nc, ident)

            for j in range(NPAIR):
                nc.gpsimd.dma_start(out=wo_sb[j],
                                    in_=wo_e[128 * j:128 * (j + 1), :])
            nc.gpsimd.dma_start(out=dm_sb, in_=dm_e[:, :])

            # v backgrounds: v0 = [v(64)|ones(32)]; v1 = [0(32)|ones(32)|v(64)]
            v03 = v0_sb.rearrange("p (t w) -> p t w", w=V0W)
            nc.vector.memset(v03[:, :, 64:96], 1.0)
            v13 = v1_sb.rearrange("p (t w) -> p t w", w=V1W)
            nc.vector.memset(v13[:, :, 0:32], 0.0)
            nc.vector.memset(v13[:, :, 32:64], 1.0)

            # ---------------- projections ----------------
            with (
                tc.tile_pool(name="xw", bufs=1) as XW,
                tc.tile_pool(name="ropew", bufs=2) as W,
                tc.tile_pool(name="pps", bufs=2, space="PSUM") as PPS,
            ):
                c1_sb = XW.tile([128, S], BF, tag="c1")
                c2_sb = XW.tile([128, S], BF, tag="c2")
                nc.gpsimd.dma_start(out=c1_sb, in_=c1_e[:, :])
                nc.gpsimd.dma_start(out=c2_sb, in_=c2_e[:, :])
                # x on the sync HWDGE queue, weights on the scalar queue, so
                # weight tiles land in parallel with the big x stream
                xt_sb, wq_sb, wk_sb, wv_sb = [], [], [], []
                for k in range(KT):
                    xq = nc.sync if k % 2 == 0 else nc.scalar
                    wqq = nc.scalar if k % 2 == 0 else nc.sync
                    xk = XW.tile([128, S], BF, tag=f"x{k}")
                    xq.dma_start(out=xk,
                                 in_=xt_e[128 * k:128 * (k + 1), :])
                    xt_sb.append(xk)
                    kk = XW.tile([128, HKV], BF, tag=f"wk{k}")
                    wqq.dma_start(out=kk,
                                  in_=wk_e[128 * k:128 * (k + 1), :])
                    wk_sb.append(kk)
                    vk = XW.tile([128, HKV], BF, tag=f"wv{k}")
                    wqq.dma_start(out=vk,
                                  in_=wv_e[128 * k:128 * (k + 1), :])
                    wv_sb.append(vk)
                    qk_ = XW.tile([128, HQ], BF, tag=f"wq{k}")
                    wqq.dma_start(out=qk_,
                                  in_=wq_e[128 * k:128 * (k + 1), :])
                    wq_sb.append(qk_)

                def rope(dst, raw):
                    # dst = raw*c1 + shuffle(raw)*c2
                    sh = W.tile([128, S], BF, tag="sh", name="sh")
                    t1 = W.tile([128, S], BF, tag="t1", name="t1")
                    nc.vector.stream_shuffle(sh, raw, SHUF)
                    nc.vector.tensor_tensor(t1, raw, c1_sb, MUL)
                    nc.vector.tensor_tensor(sh, sh, c2_sb, MUL)
                    nc.vector.tensor_tensor(dst, t1, sh, ADD)

                def rope_project(dst, w_tiles, col0):
                    # k-outer: one ldweights feeds 4 chunk matmuls
                    raw = W.tile([128, S], BF, tag="qraw", name="raw")
                    _tags = ["ppk0", "ppk1", "ppv0", "ppv1"]
                    pss = [PPS.tile([128, SQC], F32, tag=_tags[c],
                                    name=f"pq{c}", bufs=1)
                           for c in range(NCHUNK)]
                    for k in range(KT):
                        for c in range(NCHUNK):
                            nc.tensor.matmul(
                                pss[c],
                                w_tiles[k][:, col0:col0 + 128],
                                xt_sb[k][:, SQC * c:SQC * (c + 1)],
                                start=(k == 0), stop=(k == KT - 1),
                            )
                    for c in range(NCHUNK):
                        nc.scalar.copy(raw[:, SQC * c:SQC * (c + 1)], pss[c])
                    rope(dst, raw)

                # k / v_t / q0 interleaved per k-tile in two chunk passes so
                # the PE has dense work while x/w tiles stream in
                raw_k = W.tile([128, S], BF, tag="rawk", bufs=1)
                raw_q0 = W.tile([128, S], BF, tag="rawq0", bufs=1)
                raw_q1 = W.tile([128, S], BF, tag="rawq1", bufs=1)
                vt_raw = W.tile([128, S], BF, tag="rawv", bufs=1)
                for crng in ((0, 1), (2, 3)):
                    ps_k = [PPS.tile([128, SQC], F32, tag=f"ppk{i}",
                                     name=f"ppk{i}", bufs=1)
                            for i in range(2)]
                    ps_v = [PPS.tile([128, SQC], F32, tag=f"ppv{i}",
                                     name=f"ppv{i}", bufs=1)
                            for i in range(2)]
                    ps_q = [PPS.tile([128, SQC], F32, tag=f"ppq{i}",
                                     name=f"ppq{i}", bufs=1)
                            for i in range(2)]
                    ps_q1 = [PPS.tile([128, SQC], F32, tag=f"ppr{i}",
                                      name=f"ppr{i}", bufs=1)
                             for i in range(2)]
                    for k in range(KT):
                        for ci, c in enumerate(crng):
                            xs = xt_sb[k][:, SQC * c:SQC * (c + 1)]
                            nc.tensor.matmul(
                                ps_k[ci], wk_sb[k], xs,
                                start=(k == 0), stop=(k == KT - 1))
                            nc.tensor.matmul(
                                ps_v[ci], wv_sb[k], xs,
                                start=(k == 0), stop=(k == KT - 1))
                            nc.tensor.matmul(
                                ps_q[ci], wq_sb[k][:, 0:128], xs,
                                start=(k == 0), stop=(k == KT - 1))
                            nc.tensor.matmul(
                                ps_q1[ci], wq_sb[k][:, 128:256], xs,
                                start=(k == 0), stop=(k == KT - 1))
                    for ci, c in enumerate(crng):
                        sl = slice(SQC * c, SQC * (c + 1))
                        nc.scalar.copy(raw_k[:, sl], ps_k[ci])
                        nc.scalar.copy(vt_raw[:, sl], ps_v[ci])
                        nc.scalar.copy(raw_q0[:, sl], ps_q[ci])
                        nc.scalar.copy(raw_q1[:, sl], ps_q1[ci])
                rope(k_t, raw_k)
                rope(q_t[0], raw_q0)
                rope(q_t[1], raw_q1)
                for t in range(NSKT):
                    tp = PPS.tile([128, 128], BF, tag="ppk0", bufs=1)
                    nc.tensor.transpose(tp, vt_raw[:, SKT * t:SKT * (t + 1)],
                                        ident)
                    nc.scalar.copy(
                        v0_sb[:, V0W * t:V0W * t + 64], tp[:, 0:64])
                    nc.scalar.copy(
                        v1_sb[:, V1W * t + 64:V1W * t + 128], tp[:, 64:128])

                for j in range(2, NPAIR):
                    rope_project(q_t[j], wq_sb, 128 * j)

            # ---------- attention with interleaved wo fillers ----------
            with (
                tc.tile_pool(name="attw", bufs=2) as W,
                tc.tile_pool(name="stg", bufs=2) as ST,
                tc.tile_pool(name="avsb", bufs=4) as AVS,
                tc.tile_pool(name="scps", bufs=2, space="PSUM") as SCPS,
                tc.tile_pool(name="avps", bufs=1, space="PSUM") as AVPS,
                tc.tile_pool(name="wosb", bufs=3) as WB,
            ):
                # wo filler thunks: chunk c of attention drains chunk c-1's
                # wo matmuls between attention tiles, borrowing the just-freed
                # av psum bank (tag passed at pop time); leftovers at the end
                fillers = []

                def wo_chunk_thunks(c):
                    # 4 s-blocks x 4 n-slices; each thunk = 4 accum matmuls
                    # + psum->sbuf copy; last n triggers the output DMA
                    for si in range(4):
                        s = 4 * c + si
                        o_sb = WB.tile([128, DIM], BF, tag="osb", name="o_sb")

                        def mk(s=s, o_sb=o_sb):
                            for n in range(DIM // 512):
                                yield_n = n

                                def inner(tag, n=yield_n):
                                    pso = AVPS.tile([128, SQC], F32,
                                                    tag=tag, name="pso",
                                                    bufs=1)
                                    for j in range(NPAIR):
                                        nc.tensor.matmul(
                                            pso,
                                            attn[j][:, 128 * s:128 * (s + 1)],
                                            wo_sb[j][:, 512 * n:512 * (n + 1)],
                                            start=(j == 0),
                                            stop=(j == NPAIR - 1),
                                        )
                                    dst = o_sb[:, 512 * n:512 * (n + 1)]
                                    nc.vector.tensor_copy(dst, pso)
                                    if n == DIM // 512 - 1:
                                        nc.sync.dma_start(
                                            out=out_e[128 * s:128 * (s + 1), :],
                                            in_=o_sb)
                                yield inner
                        yield from mk()

                for c in range(NCHUNK):
                    glist = chunks[c]
                    if c > 0:
                        fillers.extend(wo_chunk_thunks(c - 1))
                    # pop fillers roughly evenly across pairs 1..3's tiles
                    n_slots = 3 * (len(glist) - 1)
                    per_slot = (len(fillers) + n_slots - 1) // max(n_slots, 1)

                    # denominator transpose staging: dT[p, 4j+b] = D_lo_j
                    # at q=4p+b (cols 0:16), D_hi_j at cols 16:32; one
                    # partition-parallel reciprocal covers 4 rows at once.
                    # recip rows land at partition 0 (the only source
                    # partition the gpsimd broadcast ucode supports).
                    dT = ST.tile([128, 32], F32, tag="dT", name="dT")
                    dT_r = ST.tile([128, 32], F32, tag="dTr", name="dT_r")
                    rrow_lo = ST.tile([128, NPAIR * SQC], F32, tag="rrowl",
                                      name="rrow_lo")
                    rrow_hi = ST.tile([128, NPAIR * SQC], F32, tag="rrowh",
                                      name="rrow_hi")

                    pending_norm = []

                    def half_norm(half):
                        # half 0 -> pairs 0,1 ; half 1 -> pairs 2,3
                        j0 = 2 * half
                        cl = slice(4 * j0, 4 * j0 + 8)
                        ch = slice(16 + 4 * j0, 16 + 4 * j0 + 8)
                        nc.vector.reciprocal(dT_r[:, cl], dT[:, cl])
                        nc.vector.reciprocal(dT_r[:, ch], dT[:, ch])
                        for j in (j0, j0 + 1):
                            lo_v = rrow_lo[0:1, SQC * j:SQC * (j + 1)]
                            hi_v = rrow_hi[0:1, SQC * j:SQC * (j + 1)]
                            nc.gpsimd.dma_start(
                                out=lo_v.rearrange("p (pp b) -> p pp b",
                                                   pp=128, b=4),
                                in_=dT_r[:, 4 * j:4 * j + 4])
                            nc.gpsimd.dma_start(
                                out=hi_v.rearrange("p (pp b) -> p pp b",
                                                   pp=128, b=4),
                                in_=dT_r[:, 16 + 4 * j:16 + 4 * j + 4])
                        for (j, asl, ash) in pending_norm:
                            rb_lo = W.tile([128, SQC], F32, tag="rblo",
                                           name="rb_lo")
                            rb_hi = W.tile([128, SQC], F32, tag="rbhi",
                                           name="rb_hi")
                            nc.gpsimd.partition_broadcast(
                                rb_lo, rrow_lo[0:1, SQC * j:SQC * (j + 1)])
                            nc.vector.tensor_tensor(
                                attn[j][0:64, SQC * c:SQC * (c + 1)],
                                asl[0:64, :], rb_lo[0:64, :], MUL)
                            nc.gpsimd.partition_broadcast(
                                rb_hi, rrow_hi[0:1, SQC * j:SQC * (j + 1)])
                            nc.vector.tensor_tensor(
                                attn[j][64:128, SQC * c:SQC * (c + 1)],
                                ash[64:128, :], rb_hi[64:128, :], MUL)

                    fl_cnt = 0
                    for j in range(NPAIR):
                        pp = j % 2
                        av_lo = AVPS.tile([128, SQC], F32, tag=f"avlo{pp}",
                                          name="av_lo")
                        av_hi = AVPS.tile([128, SQC], F32, tag=f"avhi{pp}",
                                          name="av_hi")
                        # software pipeline: QK/mask/exp for tile ti runs one
                        # step ahead of the AV matmuls for tile ti-1, keeping
                        # ScalarE busy while the PE works
                        pend_av = None

                        def emit_av(ent):
                            t, r, first, last, p = ent
                            nc.tensor.matmul(
                                av_lo[0:96, r:SQC],
                                v0_sb[:, V0W * t:V0W * t + V0W],
                                p[:, r:SQC],
                                start=first, stop=last,
                            )
                            nc.tensor.matmul(
                                av_hi[0:128, r:SQC],
                                v1_sb[:, V1W * t:V1W * t + V1W],
                                p[:, SQC + r:2 * SQC],
                                start=first, stop=last,
                            )

                        for ti, (t, diag, r0) in enumerate(glist):
                            first = ti == 0
                            last = ti == len(glist) - 1
                            r = 0 if first else r0
                            sc = SCPS.tile([128, 2 * SQC], F32,
                                           tag="sc", name="sc")
                            nc.tensor.matmul(
                                sc[:, r:SQC],
                                k_t[0:64, SKT * t:SKT * (t + 1)],
                                q_t[j][0:64, SQC * c + r:SQC * (c + 1)],
                                start=True, stop=not diag,
                            )
                            nc.tensor.matmul(
                                sc[:, SQC + r:2 * SQC],
                                k_t[64:128, SKT * t:SKT * (t + 1)],
                                q_t[j][64:128, SQC * c + r:SQC * (c + 1)],
                                start=True, stop=not diag,
                            )
                            if diag:
                                # additive -240 lower-triangle on the
                                # 128-wide diagonal block only
                                nc.tensor.matmul(
                                    sc[:, r0:r0 + 128], ident, dm_sb,
                                    start=False, stop=True,
                                )
                                nc.tensor.matmul(
                                    sc[:, SQC + r0:SQC + r0 + 128], ident,
                                    dm_sb, start=False, stop=True,
                                )
                            p = W.tile([128, 2 * SQC], BF, tag="p", name="p",
                                       bufs=4)
                            if r:
                                sc3 = sc.rearrange(
                                    "q (h f) -> q h f", h=2)[:, :, r:SQC]
                                p3 = p.rearrange(
                                    "q (h f) -> q h f", h=2)[:, :, r:SQC]
                                nc.scalar.activation(p3, sc3, EXP,
                                                     scale=0.125)
                            else:
                                nc.scalar.activation(p, sc, EXP, scale=0.125)
                            if pend_av is not None:
                                emit_av(pend_av)
                                # the set not in use by this pair was freed
                                # by the previous pair's copies; wo fillers
                                # borrow its banks (skip pair 0: the source
                                # attn rows may still be normalizing)
                                if j > 0:
                                    for _ in range(per_slot):
                                        if fillers:
                                            bank = ("avlo" if fl_cnt % 2 == 0
                                                    else "avhi")
                                            fillers.pop(0)(
                                                f"{bank}{(j + 1) % 2}")
                                            fl_cnt += 1
                            pend_av = (t, r, first, last, p)
                        emit_av(pend_av)
                        # evacuate av psum to sbuf (frees the bank set) and
                        # scatter this pair's denominator row into dT
                        asl = AVS.tile([128, SQC], F32, tag="avsblo",
                                       name="av_sb_lo", bufs=4)
                        ash = AVS.tile([128, SQC], F32, tag="avsbhi",
                                       name="av_sb_hi", bufs=4)
                        nc.vector.tensor_copy(asl[0:96, :], av_lo[0:96, :])
                        nc.vector.tensor_copy(ash[64:128, :],
                                              av_hi[64:128, :])
                        nc.vector.tensor_copy(ash[32:33, :],
                                              av_hi[32:33, :])
                        nc.sync.dma_start(out=dT[:, 4 * j:4 * j + 4],
                                          in_=asl[64:65, :])
                        nc.sync.dma_start(out=dT[:, 16 + 4 * j:16 + 4 * j + 4],
                                          in_=ash[32:33, :])
                        pending_norm.append((j, asl, ash))
                        if j % 2 == 1:
                            half_norm(j // 2)
                            pending_norm.clear()
                # drain remaining wo work (chunk 3)
                fillers.extend(wo_chunk_thunks(NCHUNK - 1))
                tags = ["avlo0", "avhi0", "avlo1", "avhi1"]
                for i, f in enumerate(fillers):
                    f(tags[i % 4])

    nc.finalize()
    return nc


def kernel(**inputs):
    global last_exec_time_ns, last_trace
    from concourse.bass_utils import run_bass_kernel_spmd

    x = np.asarray(inputs["x"], np.float32)
    freqs_cos = np.asarray(inputs["freqs_cos"], np.float32)
    freqs_sin = np.asarray(inputs["freqs_sin"], np.float32)
    mask = np.asarray(inputs["mask"], np.float32)
    wq = np.asarray(inputs["wq"], np.float32)
    wk = np.asarray(inputs["wk"], np.float32)
    wv = np.asarray(inputs["wv"], np.float32)
    wo = np.asarray(inputs["wo"], np.float32)

    chunks = _mask_structure(mask)
    # fixed additive mask for the diagonal 128-block: dm[k, q] = 0 if q >= k
    dmask = np.where(np.arange(128)[None, :] >= np.arange(128)[:, None],
                     0.0, -240.0).astype(BF16)

    if chunks not in _build_cache:
        _build_cache[chunks] = _build(chunks)
    nc = _build_cache[chunks]

    # trig tiles in pair layout (same for both heads of a pair)
    fi2 = np.tile(_freq, 2)
    sg2 = np.tile(_sgn, 2)
    c1 = freqs_cos.T[fi2].astype(BF16)                      # [128, S]
    c2 = (freqs_sin.T[fi2] * sg2[:, None]).astype(BF16)     # [128, S]

    # pair j holds (q-head j, q-head j+4) so lo half uses kv 0, hi half kv 1
    pair_order = [0, 4, 1, 5, 2, 6, 3, 7]
    q_cols = np.concatenate([64 * pair_order[i] + _perm
                             for i in range(H // TP)])
    o_rows = np.concatenate([np.arange(64 * pair_order[i],
                                       64 * pair_order[i] + 64)
                             for i in range(H // TP)])
    kv_perm = np.concatenate([64 * h + _perm for h in range(KV // TP)])

    in_maps = []
    for d in range(DP):
        xt = np.ascontiguousarray(x[d].T).astype(BF16)
        for t in range(TP):
            wq_s = np.ascontiguousarray(
                wq[:, HQ * t:HQ * (t + 1)][:, q_cols]).astype(BF16)
            wk_s = np.ascontiguousarray(
                wk[:, HKV * t:HKV * (t + 1)][:, kv_perm]).astype(BF16)
            wv_s = np.ascontiguousarray(
                wv[:, HKV * t:HKV * (t + 1)]).astype(BF16)
            wo_s = np.ascontiguousarray(
                wo[HQ * t:HQ * (t + 1), :][o_rows]).astype(BF16)
            in_maps.append({
                "xt": xt, "wq": wq_s, "wk": wk_s, "wv": wv_s, "wo": wo_s,
                "c1": c1, "c2": c2, "dmask": dmask,
            })

    trace = bool(os.environ.get("BASS_KERNEL_TRACE"))
    res = run_bass_kernel_spmd(nc, in_maps, core_ids=list(range(NCORES)),
                               trace=trace)
    last_exec_time_ns = res.exec_time_ns
    last_trace = res
    out = np.empty((B, S, DIM), np.float32)
    for d in range(DP):
        acc = res.results[d * TP]["out"].astype(np.float32)
        for t in range(1, TP):
            acc = acc + res.results[d * TP + t]["out"]
        out[d] = acc
    return out


# revision 22
# speedup vs baseline: 1.1045x; 1.0152x over previous
"""Trainium2 Bass kernel for GQA attention (B=2, S=2048, DIM=2048, H=32, KV=8, HD=64).

Sharding: tensor-parallel over kv heads (TP=4, 2 kv heads / 8 q heads per core)
x data-parallel over batch (DP=2).  Core c = d*4 + t.  Each core computes a
partial out = attn_out_shard @ wo_rows_shard for its batch; the host sums the
4 TP partials per batch.

All host-side work is layout-only: transpose x, permute wq/wk columns into a
RoPE-friendly even/odd layout, cast to bf16, build trig/mask pattern tiles.

Device dataflow (per core):
 - projections with x^T resident in SBUF (bf16 matmuls, k-outer accumulation)
 - RoPE via stream_shuffle + two tensor muls + add
 - attention with transposed scores (scores[sk, sq]); exp on ScalarE
 - causal masking by block skipping + a single additive -240 lower-triangle
   [128,128] pattern applied (via PE) only to the diagonal 128-block of the
   four partial tiles per chunk, with column-trimmed exp/av on those tiles
 - softmax denominators ride as ones-columns in the AV lhsT at per-pair
   distinct output partitions (lo: 64+j, hi: 60+j), staged into one SBUF
   tile per chunk so a single DVE reciprocal covers 4 rows at once;
   normalization = gpsimd partition_broadcast -> DVE mul
 - wo output projection matmuls for chunk c-1 interleaved into chunk c's
   attention tiles as PE filler work; outputs streamed to DRAM
"""

import os
import sys

import numpy as np

_REPO = "/opt/trn_rl_repo"
if _REPO not in sys.path:
    sys.path.insert(0, _REPO)

import ml_dtypes  # noqa: E402

BF16 = ml_dtypes.bfloat16

B, S, DIM = 2, 2048, 2048
H, KV, HD = 32, 8, 64
TP, DP = 4, 2
NCORES = TP * DP
HQ = (H // TP) * HD          # 512 q-proj cols per core
HKV = (KV // TP) * HD        # 128 kv-proj cols per core
NKVC = KV // TP              # 2 kv heads per core
NPAIR = (H // TP) // 2       # 4 q-head pairs per core
SQC = 512                    # sq chunk width
NCHUNK = S // SQC
SKT = 128                    # sk tile height
NSKT = S // SKT
KT = DIM // 128              # contraction tiles
V0W = 96                     # v0 tile: [v(64) | ones(32)] -> denom row 64
V1W = 128                    # v1 tile: [0(32) | ones(32) | v(64)] -> denom row 32

# RoPE layout: within each head's 64 dims -> 64 partitions, quadrant q (32)
# holds pairs 16q..16q+15 as [evens(16) | odds(16)].
_perm = np.empty(64, np.int64)
_freq = np.empty(64, np.int64)
_sgn = np.empty(64, np.float32)
for _p in range(64):
    _q, _j = divmod(_p, 32)
    if _j < 16:
        _i = 16 * _q + _j
        _perm[_p] = 2 * _i
        _sgn[_p] = -1.0
    else:
        _i = 16 * _q + _j - 16
        _perm[_p] = 2 * _i + 1
        _sgn[_p] = 1.0
    _freq[_p] = _i
SHUF = list(range(16, 32)) + list(range(0, 16))

_build_cache = {}
last_exec_time_ns = None
last_trace = None


def _mask_structure(mask):
    """chunks[c] = [(t, diag, r), ...] per valid sk tile.  diag tiles get the
    fixed lower-triangle -240 pattern added to cols [r, r+128)."""
    valid = mask[0, 0] == 0.0  # [sq, sk]
    chunks = []
    for c in range(NCHUNK):
        glist = []
        for t in range(NSKT):
            sub = valid[c * SQC:(c + 1) * SQC, t * SKT:(t + 1) * SKT]
            if not sub.any():
                continue
            if sub.all():
                glist.append((t, False, 0))
                continue
            r = int(np.argmax(sub.any(axis=1)))
            # check the partial tile is the standard causal diagonal block:
            # valid iff sq >= r + sk_within_tile
            qq, kk = np.meshgrid(np.arange(SQC), np.arange(SKT), indexing="ij")
            assert (sub == (qq >= r + kk)).all(), "non-causal partial tile"
            glist.append((t, True, r))
        # full tiles first so the first av matmul covers all columns
        glist.sort(key=lambda g: g[1])
        chunks.append(tuple(glist))
    return tuple(chunks)


def _build(chunks):
    import concourse.bass as bass  # noqa: F401
    import concourse.mybir as mybir
    from concourse import bacc
    from concourse.masks import make_identity
    from concourse.tile import TileContext

    F32, BF = mybir.dt.float32, mybir.dt.bfloat16
    MUL = mybir.AluOpType.mult
    ADD = mybir.AluOpType.add
    EXP = mybir.ActivationFunctionType.Exp

    nc = bacc.Bacc()
    xt_e = nc.declare_dram_parameter("xt", [DIM, S], BF, isOutput=False)
    wq_e = nc.declare_dram_parameter("wq", [DIM, HQ], BF, isOutput=False)
    wk_e = nc.declare_dram_parameter("wk", [DIM, HKV], BF, isOutput=False)
    wv_e = nc.declare_dram_parameter("wv", [DIM, HKV], BF, isOutput=False)
    wo_e = nc.declare_dram_parameter("wo", [HQ, DIM], BF, isOutput=False)
    c1_e = nc.declare_dram_parameter("c1", [128, S], BF, isOutput=False)
    c2_e = nc.declare_dram_parameter("c2", [128, S], BF, isOutput=False)
    dm_e = nc.declare_dram_parameter("dmask", [128, 128], BF, isOutput=False)
    out_e = nc.declare_dram_parameter("out", [S, DIM], BF, isOutput=True)

    with TileContext(nc) as tc:
        with tc.tile_pool(name="persist", bufs=1) as P:
            q_t = [P.tile([128, S], BF, tag=f"q{j}", name=f"q{j}")
                   for j in range(NPAIR)]
            k_t = P.tile([128, S], BF, tag="kt")
            v0_sb = P.tile([128, NSKT * V0W], BF, tag="v0")
            v1_sb = P.tile([128, NSKT * V1W], BF, tag="v1")
            attn = [P.tile([128, S], BF, tag=f"a{j}", name=f"a{j}")
                    for j in range(NPAIR)]
            wo_sb = [P.tile([128, DIM], BF, tag=f"wo{j}", name=f"wo{j}")
                     for j in range(NPAIR)]
            dm_sb = P.tile([128, 128], BF, tag="dm")
            ident = P.tile([128, 128], BF, tag="ident")
            make_identity(nc, ident)

            for j in range(NPAIR):
                nc.gpsimd.dma_start(out=wo_sb[j],
                                    in_=wo_e[128 * j:128 * (j + 1), :])
            nc.gpsimd.dma_start(out=dm_sb, in_=dm_e[:, :])

            # v backgrounds: v0 = [v(64)|ones(32)]; v1 = [0(32)|ones(32)|v(64)]
            v03 = v0_sb.rearrange("p (t w) -> p t w", w=V0W)
            nc.vector.memset(v03[:, :, 64:96], 1.0)
            v13 = v1_sb.rearrange("p (t w) -> p t w", w=V1W)
            nc.vector.memset(v13[:, :, 0:32], 0.0)
            nc.vector.memset(v13[:, :, 32:64], 1.0)

            # ---------------- projections ----------------
            with (
                tc.tile_pool(name="xw", bufs=1) as XW,
                tc.tile_pool(name="ropew", bufs=2) as W,
                tc.tile_pool(name="pps", bufs=2, space="PSUM") as PPS,
            ):
                c1_sb = XW.tile([128, S], BF, tag="c1")
                c2_sb = XW.tile([128, S], BF, tag="c2")
                nc.gpsimd.dma_start(out=c1_sb, in_=c1_e[:, :])
                nc.gpsimd.dma_start(out=c2_sb, in_=c2_e[:, :])
                # HAM warmup: ~36 junk matmuls while the first DMAs stream,
                # so the PE clock gate opens before real work arrives
                wps = PPS.tile([128, 128], F32, tag="ppq0", bufs=1,
                               name="warm")
                for _ in range(36):
                    nc.tensor.matmul(wps, ident, ident, start=True, stop=True)
                # x on the sync HWDGE queue, weights on the scalar queue, so
                # weight tiles land in parallel with the big x stream; x
                # arrives in column halves (the first crng pass only needs
                # cols 0:1024, so matmuls start ~2x sooner)
                xt_sb, wq_sb, wk_sb, wv_sb = [], [], [], []
                for k in range(KT):
                    xq = nc.sync if k % 2 == 0 else nc.scalar
                    wqq = nc.scalar if k % 2 == 0 else nc.sync
                    xk = XW.tile([128, S], BF, tag=f"x{k}")
                    xq.dma_start(out=xk[:, 0:1024],
                                 in_=xt_e[128 * k:128 * (k + 1), 0:1024])
                    xt_sb.append(xk)
                    kk = XW.tile([128, HKV], BF, tag=f"wk{k}")
                    wqq.dma_start(out=kk,
                                  in_=wk_e[128 * k:128 * (k + 1), :])
                    wk_sb.append(kk)
                    vk = XW.tile([128, HKV], BF, tag=f"wv{k}")
                    wqq.dma_start(out=vk,
                                  in_=wv_e[128 * k:128 * (k + 1), :])
                    wv_sb.append(vk)
                    qk_ = XW.tile([128, HQ], BF, tag=f"wq{k}")
                    wqq.dma_start(out=qk_,
                                  in_=wq_e[128 * k:128 * (k + 1), :])
                    wq_sb.append(qk_)
                for k in range(KT):
                    xq = nc.sync if k % 2 == 0 else nc.scalar
                    xq.dma_start(out=xt_sb[k][:, 1024:2048],
                                 in_=xt_e[128 * k:128 * (k + 1), 1024:2048])

                def rope(dst, raw):
                    # dst = raw*c1 + shuffle(raw)*c2
                    sh = W.tile([128, S], BF, tag="sh", name="sh")
                    t1 = W.tile([128, S], BF, tag="t1", name="t1")
                    nc.vector.stream_shuffle(sh, raw, SHUF)
                    nc.vector.tensor_tensor(t1, raw, c1_sb, MUL)
                    nc.vector.tensor_tensor(sh, sh, c2_sb, MUL)
                    nc.vector.tensor_tensor(dst, t1, sh, ADD)

                def rope_project(dst, w_tiles, col0):
                    # k-outer: one ldweights feeds 4 chunk matmuls
                    raw = W.tile([128, S], BF, tag="qraw", name="raw")
                    _tags = ["ppk0", "ppk1", "ppv0", "ppv1"]
                    pss = [PPS.tile([128, SQC], F32, tag=_tags[c],
                                    name=f"pq{c}", bufs=1)
                           for c in range(NCHUNK)]
                    for k in range(KT):
                        for c in range(NCHUNK):
                            nc.tensor.matmul(
                                pss[c],
                                w_tiles[k][:, col0:col0 + 128],
                                xt_sb[k][:, SQC * c:SQC * (c + 1)],
                                start=(k == 0), stop=(k == KT - 1),
                            )
                    for c in range(NCHUNK):
                        # split psum->sbuf copies across ACT and DVE so the
                        # ACT queue is clear for attention's exps
                        if c % 2 == 0:
                            nc.vector.tensor_copy(
                                raw[:, SQC * c:SQC * (c + 1)], pss[c])
                        else:
                            nc.scalar.copy(raw[:, SQC * c:SQC * (c + 1)],
                                           pss[c])
                    rope(dst, raw)

                # k / v_t / q0 interleaved per k-tile in two chunk passes so
                # the PE has dense work while x/w tiles stream in
                raw_k = W.tile([128, S], BF, tag="rawk", bufs=1)
                raw_q0 = W.tile([128, S], BF, tag="rawq0", bufs=1)
                raw_q1 = W.tile([128, S], BF, tag="rawq1", bufs=1)
                vt_raw = W.tile([128, S], BF, tag="rawv", bufs=1)
                for crng in ((0, 1), (2, 3)):
                    ps_k = [PPS.tile([128, SQC], F32, tag=f"ppk{i}",
                                     name=f"ppk{i}", bufs=1)
                            for i in range(2)]
                    ps_v = [PPS.tile([128, SQC], F32, tag=f"ppv{i}",
                                     name=f"ppv{i}", bufs=1)
                            for i in range(2)]
                    ps_q = [PPS.tile([128, SQC], F32, tag=f"ppq{i}",
                                     name=f"ppq{i}", bufs=1)
                            for i in range(2)]
                    ps_q1 = [PPS.tile([128, SQC], F32, tag=f"ppr{i}",
                                      name=f"ppr{i}", bufs=1)
                             for i in range(2)]
                    for k in range(KT):
                        for ci, c in enumerate(crng):
                            xs = xt_sb[k][:, SQC * c:SQC * (c + 1)]
                            nc.tensor.matmul(
                                ps_k[ci], wk_sb[k], xs,
                                start=(k == 0), stop=(k == KT - 1))
                            nc.tensor.matmul(
                                ps_v[ci], wv_sb[k], xs,
                                start=(k == 0), stop=(k == KT - 1))
                            nc.tensor.matmul(
                                ps_q[ci], wq_sb[k][:, 0:128], xs,
                                start=(k == 0), stop=(k == KT - 1))
                            nc.tensor.matmul(
                                ps_q1[ci], wq_sb[k][:, 128:256], xs,
                                start=(k == 0), stop=(k == KT - 1))
                    for ci, c in enumerate(crng):
                        sl = slice(SQC * c, SQC * (c + 1))
                        nc.scalar.copy(raw_k[:, sl], ps_k[ci])
                        nc.scalar.copy(vt_raw[:, sl], ps_v[ci])
                        nc.scalar.copy(raw_q0[:, sl], ps_q[ci])
                        nc.scalar.copy(raw_q1[:, sl], ps_q1[ci])
                rope(k_t, raw_k)
                rope(q_t[0], raw_q0)
                rope(q_t[1], raw_q1)
                for t in range(NSKT):
                    tp = PPS.tile([128, 128], BF, tag="ppk0", bufs=1)
                    nc.tensor.transpose(tp, vt_raw[:, SKT * t:SKT * (t + 1)],
                                        ident)
                    nc.scalar.copy(
                        v0_sb[:, V0W * t:V0W * t + 64], tp[:, 0:64])
                    nc.scalar.copy(
                        v1_sb[:, V1W * t + 64:V1W * t + 128], tp[:, 64:128])

                for j in range(2, NPAIR):
                    rope_project(q_t[j], wq_sb, 128 * j)

            # ---------- attention with interleaved wo fillers ----------
            with (
                tc.tile_pool(name="attw", bufs=2) as W,
                tc.tile_pool(name="stg", bufs=2) as ST,
                tc.tile_pool(name="avsb", bufs=4) as AVS,
                tc.tile_pool(name="scps", bufs=2, space="PSUM") as SCPS,
                tc.tile_pool(name="avps", bufs=1, space="PSUM") as AVPS,
                tc.tile_pool(name="wosb", bufs=3) as WB,
            ):
                # wo filler thunks: chunk c of attention drains chunk c-1's
                # wo matmuls between attention tiles, borrowing the just-freed
                # av psum bank (tag passed at pop time); leftovers at the end
                fillers = []

                def wo_chunk_thunks(c):
                    # 4 s-blocks x 4 n-slices; each thunk = 4 accum matmuls
                    # + psum->sbuf copy; last n triggers the output DMA
                    for si in range(4):
                        s = 4 * c + si
                        o_sb = WB.tile([128, DIM], BF, tag="osb", name="o_sb")

                        def mk(s=s, o_sb=o_sb):
                            for n in range(DIM // 512):
                                yield_n = n

                                def inner(tag, n=yield_n):
                                    pso = AVPS.tile([128, SQC], F32,
                                                    tag=tag, name="pso",
                                                    bufs=1)
                                    for j in range(NPAIR):
                                        nc.tensor.matmul(
                                            pso,
                                            attn[j][:, 128 * s:128 * (s + 1)],
                                            wo_sb[j][:, 512 * n:512 * (n + 1)],
                                            start=(j == 0),
                                            stop=(j == NPAIR - 1),
                                        )
                                    dst = o_sb[:, 512 * n:512 * (n + 1)]
                                    nc.vector.tensor_copy(dst, pso)
                                    if n == DIM // 512 - 1:
                                        nc.sync.dma_start(
                                            out=out_e[128 * s:128 * (s + 1), :],
                                            in_=o_sb)
                                yield inner
                        yield from mk()

                for c in range(NCHUNK):
                    glist = chunks[c]
                    if c > 0:
                        fillers.extend(wo_chunk_thunks(c - 1))
                    # pop fillers roughly evenly across pairs 1..3's tiles
                    n_slots = 3 * (len(glist) - 1)
                    per_slot = (len(fillers) + n_slots - 1) // max(n_slots, 1)

                    # denominator transpose staging: dT[p, 4j+b] = D_lo_j
                    # at q=4p+b (cols 0:16), D_hi_j at cols 16:32; one
                    # partition-parallel reciprocal covers 4 rows at once.
                    # recip rows land at partition 0 (the only source
                    # partition the gpsimd broadcast ucode supports).
                    dT = ST.tile([128, 32], F32, tag="dT", name="dT")
                    dT_r = ST.tile([128, 32], F32, tag="dTr", name="dT_r")
                    rrow_lo = ST.tile([128, NPAIR * SQC], F32, tag="rrowl",
                                      name="rrow_lo")
                    rrow_hi = ST.tile([128, NPAIR * SQC], F32, tag="rrowh",
                                      name="rrow_hi")

                    def pair_norm(j, asl, ash):
                        nc.vector.reciprocal(dT_r[:, 4 * j:4 * j + 4],
                                             dT[:, 4 * j:4 * j + 4])
                        nc.vector.reciprocal(
                            dT_r[:, 16 + 4 * j:16 + 4 * j + 4],
                            dT[:, 16 + 4 * j:16 + 4 * j + 4])
                        lo_v = rrow_lo[0:1, SQC * j:SQC * (j + 1)]
                        hi_v = rrow_hi[0:1, SQC * j:SQC * (j + 1)]
                        nc.gpsimd.dma_start(
                            out=lo_v.rearrange("p (pp b) -> p pp b",
                                               pp=128, b=4),
                            in_=dT_r[:, 4 * j:4 * j + 4])
                        nc.gpsimd.dma_start(
                            out=hi_v.rearrange("p (pp b) -> p pp b",
                                               pp=128, b=4),
                            in_=dT_r[:, 16 + 4 * j:16 + 4 * j + 4])
                        rb_lo = W.tile([128, SQC], F32, tag="rblo",
                                       name="rb_lo")
                        rb_hi = W.tile([128, SQC], F32, tag="rbhi",
                                       name="rb_hi")
                        nc.gpsimd.partition_broadcast(
                            rb_lo, rrow_lo[0:1, SQC * j:SQC * (j + 1)])
                        nc.vector.tensor_tensor(
                            attn[j][0:64, SQC * c:SQC * (c + 1)],
                            asl[0:64, :], rb_lo[0:64, :], MUL)
                        nc.gpsimd.partition_broadcast(
                            rb_hi, rrow_hi[0:1, SQC * j:SQC * (j + 1)])
                        nc.vector.tensor_tensor(
                            attn[j][64:128, SQC * c:SQC * (c + 1)],
                            ash[64:128, :], rb_hi[64:128, :], MUL)

                    fl_cnt = 0
                    for j in range(NPAIR):
                        pp = j % 2
                        av_lo = AVPS.tile([128, SQC], F32, tag=f"avlo{pp}",
                                          name="av_lo")
                        av_hi = AVPS.tile([128, SQC], F32, tag=f"avhi{pp}",
                                          name="av_hi")
                        # software pipeline: QK/mask/exp for tile ti runs one
                        # step ahead of the AV matmuls for tile ti-1, keeping
                        # ScalarE busy while the PE works
                        pend_av = None

                        def emit_av(ent):
                            t, r, first, last, p = ent
                            nc.tensor.matmul(
                                av_lo[0:96, r:SQC],
                                v0_sb[:, V0W * t:V0W * t + V0W],
                                p[:, r:SQC],
                                start=first, stop=last,
                            )
                            nc.tensor.matmul(
                                av_hi[0:128, r:SQC],
                                v1_sb[:, V1W * t:V1W * t + V1W],
                                p[:, SQC + r:2 * SQC],
                                start=first, stop=last,
                            )

                        for ti, (t, diag, r0) in enumerate(glist):
                            first = ti == 0
                            last = ti == len(glist) - 1
                            r = 0 if first else r0
                            sc = SCPS.tile([128, 2 * SQC], F32,
                                           tag="sc", name="sc")
                            nc.tensor.matmul(
                                sc[:, r:SQC],
                                k_t[0:64, SKT * t:SKT * (t + 1)],
                                q_t[j][0:64, SQC * c + r:SQC * (c + 1)],
                                start=True, stop=not diag,
                            )
                            nc.tensor.matmul(
                                sc[:, SQC + r:2 * SQC],
                                k_t[64:128, SKT * t:SKT * (t + 1)],
                                q_t[j][64:128, SQC * c + r:SQC * (c + 1)],
                                start=True, stop=not diag,
                            )
                            if diag:
                                # additive -240 lower-triangle on the
                                # 128-wide diagonal block only
                                nc.tensor.matmul(
                                    sc[:, r0:r0 + 128], ident, dm_sb,
                                    start=False, stop=True,
                                )
                                nc.tensor.matmul(
                                    sc[:, SQC + r0:SQC + r0 + 128], ident,
                                    dm_sb, start=False, stop=True,
                                )
                            p = W.tile([128, 2 * SQC], BF, tag="p", name="p",
                                       bufs=4)
                            if r:
                                sc3 = sc.rearrange(
                                    "q (h f) -> q h f", h=2)[:, :, r:SQC]
                                p3 = p.rearrange(
                                    "q (h f) -> q h f", h=2)[:, :, r:SQC]
                                nc.scalar.activation(p3, sc3, EXP,
                                                     scale=0.125)
                            else:
                                nc.scalar.activation(p, sc, EXP, scale=0.125)
                            if pend_av is not None:
                                emit_av(pend_av)
                                # the set not in use by this pair was freed
                                # by the previous pair's copies; wo fillers
                                # borrow its banks (skip pair 0: the source
                                # attn rows may still be normalizing)
                                if j > 0:
                                    for _ in range(per_slot):
                                        if fillers:
                                            bank = ("avlo" if fl_cnt % 2 == 0
                                                    else "avhi")
                                            fillers.pop(0)(
                                                f"{bank}{(j + 1) % 2}")
                                            fl_cnt += 1
                            pend_av = (t, r, first, last, p)
                        emit_av(pend_av)
                        # evacuate av psum to sbuf (frees the bank set) and
                        # scatter this pair's denominator row into dT
                        asl = AVS.tile([128, SQC], F32, tag="avsblo",
                                       name="av_sb_lo", bufs=4)
                        ash = AVS.tile([128, SQC], F32, tag="avsbhi",
                                       name="av_sb_hi", bufs=4)
                        nc.vector.tensor_copy(asl[0:96, :], av_lo[0:96, :])
                        nc.vector.tensor_copy(ash[64:128, :],
                                              av_hi[64:128, :])
                        nc.vector.tensor_copy(ash[32:33, :],
                                              av_hi[32:33, :])
                        nc.sync.dma_start(out=dT[:, 4 * j:4 * j + 4],
                                          in_=asl[64:65, :])
                        nc.sync.dma_start(out=dT[:, 16 + 4 * j:16 + 4 * j + 4],
                                          in_=ash[32:33, :])
                        pair_norm(j, asl, ash)
                # drain remaining wo work (chunk 3)
                fillers.extend(wo_chunk_thunks(NCHUNK - 1))
                tags = ["avlo0", "avhi0", "avlo1", "avhi1"]
                for i, f in enumerate(fillers):
                    f(tags[i % 4])

    nc.finalize()
    return nc


def kernel(**inputs):
    global last_exec_time_ns, last_trace
    from concourse.bass_utils import run_bass_kernel_spmd

    x = np.asarray(inputs["x"], np.float32)
    freqs_cos = np.asarray(inputs["freqs_cos"], np.float32)
    freqs_sin = np.asarray(inputs["freqs_sin"], np.float32)
    mask = np.asarray(inputs["mask"], np.float32)
    wq = np.asarray(inputs["wq"], np.float32)
    wk = np.asarray(inputs["wk"], np.float32)
    wv = np.asarray(inputs["wv"], np.float32)
    wo = np.asarray(inputs["wo"], np.float32)

    chunks = _mask_structure(mask)
    # fixed additive mask for the diagonal 128-block: dm[k, q] = 0 if q >= k
    dmask = np.where(np.arange(128)[None, :] >= np.arange(128)[:, None],
                     0.0, -240.0).astype(BF16)

    if chunks not in _build_cache:
        _build_cache[chunks] = _build(chunks)
    nc = _build_cache[chunks]

    # trig tiles in pair layout (same for both heads of a pair)
    fi2 = np.tile(_freq, 2)
    sg2 = np.tile(_sgn, 2)
    c1 = freqs_cos.T[fi2].astype(BF16)                      # [128, S]
    c2 = (freqs_sin.T[fi2] * sg2[:, None]).astype(BF16)     # [128, S]

    # pair j holds (q-head j, q-head j+4) so lo half uses kv 0, hi half kv 1
    pair_order = [0, 4, 1, 5, 2, 6, 3, 7]
    q_cols = np.concatenate([64 * pair_order[i] + _perm
                             for i in range(H // TP)])
    o_rows = np.concatenate([np.arange(64 * pair_order[i],
                                       64 * pair_order[i] + 64)
                             for i in range(H // TP)])
    kv_perm = np.concatenate([64 * h + _perm for h in range(KV // TP)])

    in_maps = []
    for d in range(DP):
        xt = np.ascontiguousarray(x[d].T).astype(BF16)
        for t in range(TP):
            wq_s = np.ascontiguousarray(
                wq[:, HQ * t:HQ * (t + 1)][:, q_cols]).astype(BF16)
            wk_s = np.ascontiguousarray(
                wk[:, HKV * t:HKV * (t + 1)][:, kv_perm]).astype(BF16)
            wv_s = np.ascontiguousarray(
                wv[:, HKV * t:HKV * (t + 1)]).astype(BF16)
            wo_s = np.ascontiguousarray(
                wo[HQ * t:HQ * (t + 1), :][o_rows]).astype(BF16)
            in_maps.append({
                "xt": xt, "wq": wq_s, "wk": wk_s, "wv": wv_s, "wo": wo_s,
                "c1": c1, "c2": c2, "dmask": dmask,
            })

    trace = bool(os.environ.get("BASS_KERNEL_TRACE"))
    res = run_bass_kernel_spmd(nc, in_maps, core_ids=list(range(NCORES)),
                               trace=trace)
    last_exec_time_ns = res.exec_time_ns
    last_trace = res
    out = np.empty((B, S, DIM), np.float32)
    for d in range(DP):
        acc = res.results[d * TP]["out"].astype(np.float32)
        for t in range(1, TP):
            acc = acc + res.results[d * TP + t]["out"]
        out[d] = acc
    return out


# revision 33
# speedup vs baseline: 1.1650x; 1.0548x over previous
"""Trainium2 Bass kernel for GQA attention (B=2, S=2048, DIM=2048, H=32, KV=8, HD=64).

Sharding: tensor-parallel over kv heads (TP=4, 2 kv heads / 8 q heads per core)
x data-parallel over batch (DP=2).  Core c = d*4 + t.  Each core computes a
partial out = attn_out_shard @ wo_rows_shard for its batch; the host sums the
4 TP partials per batch.

All host-side work is layout-only: transpose x, permute wq/wk columns into a
RoPE-friendly even/odd layout, cast to bf16, build trig/mask pattern tiles.

Device dataflow (per core):
 - projections with x^T resident in SBUF (bf16 matmuls, k-outer accumulation)
 - RoPE via stream_shuffle + two tensor muls + add
 - attention with transposed scores (scores[sk, sq]); exp on ScalarE
 - causal masking by block skipping + a single additive -240 lower-triangle
   [128,128] pattern applied (via PE) only to the diagonal 128-block of the
   four partial tiles per chunk, with column-trimmed exp/av on those tiles
 - softmax denominators ride as ones-columns in the AV lhsT at per-pair
   distinct output partitions (lo: 64+j, hi: 60+j), staged into one SBUF
   tile per chunk so a single DVE reciprocal covers 4 rows at once;
   normalization = gpsimd partition_broadcast -> DVE mul
 - wo output projection matmuls for chunk c-1 interleaved into chunk c's
   attention tiles as PE filler work; outputs streamed to DRAM
"""

import os
import sys

import numpy as np

_REPO = "/opt/trn_rl_repo"
if _REPO not in sys.path:
    sys.path.insert(0, _REPO)

import ml_dtypes  # noqa: E402

BF16 = ml_dtypes.bfloat16

B, S, DIM = 2, 2048, 2048
H, KV, HD = 32, 8, 64
TP, DP = 4, 2
NCORES = TP * DP
HQ = (H // TP) * HD          # 512 q-proj cols per core
HKV = (KV // TP) * HD        # 128 kv-proj cols per core
NKVC = KV // TP              # 2 kv heads per core
NPAIR = (H // TP) // 2       # 4 q-head pairs per core
SQC = 512                    # sq chunk width
NCHUNK = S // SQC
SKT = 128                    # sk tile height
NSKT = S // SKT
KT = DIM // 128              # contraction tiles
V0W = 96                     # v0 tile: [v(64) | ones(32)] -> denom row 64
V1W = 128                    # v1 tile: [0(32) | ones(32) | v(64)] -> denom row 32

# RoPE layout: within each head's 64 dims -> 64 partitions, quadrant q (32)
# holds pairs 16q..16q+15 as [evens(16) | odds(16)].
_perm = np.empty(64, np.int64)
_freq = np.empty(64, np.int64)
_sgn = np.empty(64, np.float32)
for _p in range(64):
    _q, _j = divmod(_p, 32)
    if _j < 16:
        _i = 16 * _q + _j
        _perm[_p] = 2 * _i
        _sgn[_p] = -1.0
    else:
        _i = 16 * _q + _j - 16
        _perm[_p] = 2 * _i + 1
        _sgn[_p] = 1.0
    _freq[_p] = _i
SHUF = list(range(16, 32)) + list(range(0, 16))

MASK_ON_DVE = False          # False: additive -240 mask via PE matmuls
# (the DVE 0/1-triangle variant mis-schedules: split-writer p regions hit a
# schedule-sensitive missing dependency; keep the PE additive mask)

_build_cache = {}
last_exec_time_ns = None
last_trace = None


def _mask_structure(mask):
    """chunks[c] = [(t, diag, r), ...] per valid sk tile.  diag tiles get the
    fixed lower-triangle -240 pattern added to cols [r, r+128)."""
    valid = mask[0, 0] == 0.0  # [sq, sk]
    chunks = []
    for c in range(NCHUNK):
        glist = []
        for t in range(NSKT):
            sub = valid[c * SQC:(c + 1) * SQC, t * SKT:(t + 1) * SKT]
            if not sub.any():
                continue
            if sub.all():
                glist.append((t, False, 0))
                continue
            r = int(np.argmax(sub.any(axis=1)))
            # check the partial tile is the standard causal diagonal block:
            # valid iff sq >= r + sk_within_tile
            qq, kk = np.meshgrid(np.arange(SQC), np.arange(SKT), indexing="ij")
            assert (sub == (qq >= r + kk)).all(), "non-causal partial tile"
            glist.append((t, True, r))
        # full tiles first so the first av matmul covers all columns
        glist.sort(key=lambda g: g[1])
        chunks.append(tuple(glist))
    return tuple(chunks)


def _build(chunks):
    import concourse.bass as bass  # noqa: F401
    import concourse.mybir as mybir
    from concourse import bacc
    from concourse.masks import make_identity
    from concourse.tile import TileContext

    F32, BF = mybir.dt.float32, mybir.dt.bfloat16
    MUL = mybir.AluOpType.mult
    ADD = mybir.AluOpType.add
    EXP = mybir.ActivationFunctionType.Exp

    nc = bacc.Bacc()
    xt_e = nc.declare_dram_parameter("xt", [DIM, S], BF, isOutput=False)
    wq_e = nc.declare_dram_parameter("wq", [DIM, HQ], BF, isOutput=False)
    wk_e = nc.declare_dram_parameter("wk", [DIM, HKV], BF, isOutput=False)
    wv_e = nc.declare_dram_parameter("wv", [DIM, HKV], BF, isOutput=False)
    wo_e = nc.declare_dram_parameter("wo", [HQ, DIM], BF, isOutput=False)
    c1_e = nc.declare_dram_parameter("c1", [128, S], BF, isOutput=False)
    c2_e = nc.declare_dram_parameter("c2", [128, S], BF, isOutput=False)
    dm_e = nc.declare_dram_parameter("dmask", [128, 128], BF, isOutput=False)
    out_e = nc.declare_dram_parameter("out", [S, DIM], BF, isOutput=True)

    with TileContext(nc) as tc:
        with tc.tile_pool(name="persist", bufs=1) as P:
            q_t = [P.tile([128, S], BF, tag=f"q{j}", name=f"q{j}")
                   for j in range(NPAIR)]
            k_t = P.tile([128, S], BF, tag="kt")
            v0_sb = P.tile([128, NSKT * V0W], BF, tag="v0")
            v1_sb = P.tile([128, NSKT * V1W], BF, tag="v1")
            attn = [P.tile([128, S], BF, tag=f"a{j}", name=f"a{j}")
                    for j in range(NPAIR)]
            wo_sb = [P.tile([128, DIM], BF, tag=f"wo{j}", name=f"wo{j}")
                     for j in range(NPAIR)]
            dm_sb = P.tile([128, 128], BF, tag="dm")
            ident = P.tile([128, 128], BF, tag="ident")
            make_identity(nc, ident)

            for j in range(NPAIR):
                nc.gpsimd.dma_start(out=wo_sb[j],
                                    in_=wo_e[128 * j:128 * (j + 1), :])
            nc.gpsimd.dma_start(out=dm_sb, in_=dm_e[:, :])

            # v backgrounds: v0 = [v(64)|ones(32)]; v1 = [0(32)|ones(32)|v(64)]
            v03 = v0_sb.rearrange("p (t w) -> p t w", w=V0W)
            nc.vector.memset(v03[:, :, 64:96], 1.0)
            v13 = v1_sb.rearrange("p (t w) -> p t w", w=V1W)
            nc.vector.memset(v13[:, :, 0:32], 0.0)
            nc.vector.memset(v13[:, :, 32:64], 1.0)

            # ---------------- projections ----------------
            with (
                tc.tile_pool(name="xw", bufs=1) as XW,
                tc.tile_pool(name="ropew", bufs=2) as W,
                tc.tile_pool(name="pps", bufs=2, space="PSUM") as PPS,
            ):
                c1_sb = XW.tile([128, S], BF, tag="c1")
                c2_sb = XW.tile([128, S], BF, tag="c2")
                nc.gpsimd.dma_start(out=c1_sb, in_=c1_e[:, :])
                nc.gpsimd.dma_start(out=c2_sb, in_=c2_e[:, :])
                # HAM warmup: ~36 junk matmuls while the first DMAs stream,
                # so the PE clock gate opens before real work arrives
                wps = PPS.tile([128, 128], F32, tag="ppq0", bufs=1,
                               name="warm")
                for _ in range(36):
                    nc.tensor.matmul(wps, ident, ident, start=True, stop=True)
                # x on the sync HWDGE queue, weights on the scalar queue, so
                # weight tiles land in parallel with the big x stream; x
                # arrives in column halves (the first crng pass only needs
                # cols 0:1024, so matmuls start ~2x sooner)
                xt_sb, wq_sb, wk_sb, wv_sb = [], [], [], []
                for k in range(KT):
                    xq = nc.sync if k % 2 == 0 else nc.scalar
                    wqq = nc.scalar if k % 2 == 0 else nc.sync
                    xk = XW.tile([128, S], BF, tag=f"x{k}")
                    xq.dma_start(out=xk[:, 0:1024],
                                 in_=xt_e[128 * k:128 * (k + 1), 0:1024])
                    nc.gpsimd.dma_start(
                        out=xk[:, 1024:2048],
                        in_=xt_e[128 * k:128 * (k + 1), 1024:2048])
                    xt_sb.append(xk)
                    kk = XW.tile([128, HKV], BF, tag=f"wk{k}")
                    wqq.dma_start(out=kk,
                                  in_=wk_e[128 * k:128 * (k + 1), :])
                    wk_sb.append(kk)
                    vk = XW.tile([128, HKV], BF, tag=f"wv{k}")
                    wqq.dma_start(out=vk,
                                  in_=wv_e[128 * k:128 * (k + 1), :])
                    wv_sb.append(vk)
                    qk_ = XW.tile([128, HQ], BF, tag=f"wq{k}")
                    wqq.dma_start(out=qk_,
                                  in_=wq_e[128 * k:128 * (k + 1), :])
                    wq_sb.append(qk_)

                def rope(dst, raw):
                    # dst = raw*c1 + shuffle(raw)*c2
                    sh = W.tile([128, S], BF, tag="sh", name="sh")
                    t1 = W.tile([128, S], BF, tag="t1", name="t1")
                    nc.vector.stream_shuffle(sh, raw, SHUF)
                    nc.vector.tensor_tensor(t1, raw, c1_sb, MUL)
                    nc.vector.tensor_tensor(sh, sh, c2_sb, MUL)
                    nc.vector.tensor_tensor(dst, t1, sh, ADD)

                def rope_project(dst, w_tiles, col0):
                    # k-outer: one ldweights feeds 4 chunk matmuls
                    raw = W.tile([128, S], BF, tag="qraw", name="raw")
                    _tags = ["ppk0", "ppk1", "ppv0", "ppv1"]
                    pss = [PPS.tile([128, SQC], F32, tag=_tags[c],
                                    name=f"pq{c}", bufs=1)
                           for c in range(NCHUNK)]
                    for k in range(KT):
                        for c in range(NCHUNK):
                            nc.tensor.matmul(
                                pss[c],
                                w_tiles[k][:, col0:col0 + 128],
                                xt_sb[k][:, SQC * c:SQC * (c + 1)],
                                start=(k == 0), stop=(k == KT - 1),
                            )
                    for c in range(NCHUNK):
                        # split psum->sbuf copies across ACT and DVE so the
                        # ACT queue is clear for attention's exps
                        if c % 2 == 0:
                            nc.vector.tensor_copy(
                                raw[:, SQC * c:SQC * (c + 1)], pss[c])
                        else:
                            nc.scalar.copy(raw[:, SQC * c:SQC * (c + 1)],
                                           pss[c])
                    rope(dst, raw)

                # k / v_t / q0 interleaved per k-tile in two chunk passes so
                # the PE has dense work while x/w tiles stream in
                raw_k = W.tile([128, S], BF, tag="rawk", bufs=1)
                raw_q0 = W.tile([128, S], BF, tag="rawq0", bufs=1)
                raw_q1 = W.tile([128, S], BF, tag="rawq1", bufs=1)
                vt_raw = W.tile([128, S], BF, tag="rawv", bufs=1)
                for crng in ((0, 1), (2, 3)):
                    ps_k = [PPS.tile([128, SQC], F32, tag=f"ppk{i}",
                                     name=f"ppk{i}", bufs=1)
                            for i in range(2)]
                    ps_v = [PPS.tile([128, SQC], F32, tag=f"ppv{i}",
                                     name=f"ppv{i}", bufs=1)
                            for i in range(2)]
                    ps_q = [PPS.tile([128, SQC], F32, tag=f"ppq{i}",
                                     name=f"ppq{i}", bufs=1)
                            for i in range(2)]
                    ps_q1 = [PPS.tile([128, SQC], F32, tag=f"ppr{i}",
                                      name=f"ppr{i}", bufs=1)
                             for i in range(2)]
                    for k in range(KT):
                        for ci, c in enumerate(crng):
                            xs = xt_sb[k][:, SQC * c:SQC * (c + 1)]
                            nc.tensor.matmul(
                                ps_k[ci], wk_sb[k], xs,
                                start=(k == 0), stop=(k == KT - 1))
                            nc.tensor.matmul(
                                ps_v[ci], wv_sb[k], xs,
                                start=(k == 0), stop=(k == KT - 1))
                            nc.tensor.matmul(
                                ps_q[ci], wq_sb[k][:, 0:128], xs,
                                start=(k == 0), stop=(k == KT - 1))
                            nc.tensor.matmul(
                                ps_q1[ci], wq_sb[k][:, 128:256], xs,
                                start=(k == 0), stop=(k == KT - 1))
                    for ci, c in enumerate(crng):
                        sl = slice(SQC * c, SQC * (c + 1))
                        nc.scalar.copy(raw_k[:, sl], ps_k[ci])
                        nc.scalar.copy(vt_raw[:, sl], ps_v[ci])
                        nc.scalar.copy(raw_q0[:, sl], ps_q[ci])
                        nc.scalar.copy(raw_q1[:, sl], ps_q1[ci])
                rope(k_t, raw_k)
                rope(q_t[0], raw_q0)
                rope(q_t[1], raw_q1)
                for t in range(NSKT):
                    tp = PPS.tile([128, 128], BF, tag="ppk0", bufs=1)
                    nc.tensor.transpose(tp, vt_raw[:, SKT * t:SKT * (t + 1)],
                                        ident)
                    nc.scalar.copy(
                        v0_sb[:, V0W * t:V0W * t + 64], tp[:, 0:64])
                    nc.scalar.copy(
                        v1_sb[:, V1W * t + 64:V1W * t + 128], tp[:, 64:128])

                for j in range(2, NPAIR):
                    rope_project(q_t[j], wq_sb, 128 * j)

            # ---------- attention with interleaved wo fillers ----------
            with (
                tc.tile_pool(name="attw", bufs=2) as W,
                tc.tile_pool(name="stg", bufs=2) as ST,
                tc.tile_pool(name="avsb", bufs=4) as AVS,
                tc.tile_pool(name="scps", bufs=2, space="PSUM") as SCPS,
                tc.tile_pool(name="avps", bufs=1, space="PSUM") as AVPS,
                tc.tile_pool(name="wosb", bufs=3) as WB,
            ):
                # wo filler thunks: chunk c of attention drains chunk c-1's
                # wo matmuls between attention tiles, borrowing the just-freed
                # av psum bank (tag passed at pop time); leftovers at the end
                fillers = []

                def wo_chunk_thunks(c):
                    # 4 s-blocks x 4 n-slices; each thunk = 4 accum matmuls
                    # + psum->sbuf copy; last n triggers the output DMA
                    for si in range(4):
                        s = 4 * c + si
                        o_sb = WB.tile([128, DIM], BF, tag="osb", name="o_sb")

                        def mk(s=s, o_sb=o_sb):
                            for n in range(DIM // 512):
                                yield_n = n

                                def inner(tag, n=yield_n):
                                    pso = AVPS.tile([128, SQC], F32,
                                                    tag=tag, name="pso",
                                                    bufs=1)
                                    for j in range(NPAIR):
                                        nc.tensor.matmul(
                                            pso,
                                            attn[j][:, 128 * s:128 * (s + 1)],
                                            wo_sb[j][:, 512 * n:512 * (n + 1)],
                                            start=(j == 0),
                                            stop=(j == NPAIR - 1),
                                        )
                                    dst = o_sb[:, 512 * n:512 * (n + 1)]
                                    nc.vector.tensor_copy(dst, pso)
                                    if n == DIM // 512 - 1:
                                        nc.sync.dma_start(
                                            out=out_e[128 * s:128 * (s + 1), :],
                                            in_=o_sb)
                                yield inner
                        yield from mk()

                for c in range(NCHUNK):
                    glist = chunks[c]
                    if c > 0:
                        fillers.extend(wo_chunk_thunks(c - 1))
                    # pop fillers evenly across pairs 1..3's tiles
                    n_slots = 3 * (len(glist) - 1)
                    fill_rate = len(fillers) / max(n_slots, 1)
                    fill_acc = 0.0

                    # denominator transpose staging: dT[p, 4j+b] = D_lo_j
                    # at q=4p+b (cols 0:16), D_hi_j at cols 16:32; one
                    # partition-parallel reciprocal covers 4 rows at once.
                    # recip rows land at partition 0 (the only source
                    # partition the gpsimd broadcast ucode supports).
                    dT = ST.tile([128, 32], F32, tag="dT", name="dT")
                    dT_r = ST.tile([128, 32], F32, tag="dTr", name="dT_r")
                    rrow_lo = ST.tile([128, NPAIR * SQC], F32, tag="rrowl",
                                      name="rrow_lo")
                    rrow_hi = ST.tile([128, NPAIR * SQC], F32, tag="rrowh",
                                      name="rrow_hi")

                    def pair_norm(j, asl, ash):
                        nc.vector.reciprocal(dT_r[:, 4 * j:4 * j + 4],
                                             dT[:, 4 * j:4 * j + 4])
                        nc.vector.reciprocal(
                            dT_r[:, 16 + 4 * j:16 + 4 * j + 4],
                            dT[:, 16 + 4 * j:16 + 4 * j + 4])
                        lo_v = rrow_lo[0:1, SQC * j:SQC * (j + 1)]
                        hi_v = rrow_hi[0:1, SQC * j:SQC * (j + 1)]
                        nc.gpsimd.dma_start(
                            out=lo_v.rearrange("p (pp b) -> p pp b",
                                               pp=128, b=4),
                            in_=dT_r[:, 4 * j:4 * j + 4])
                        nc.gpsimd.dma_start(
                            out=hi_v.rearrange("p (pp b) -> p pp b",
                                               pp=128, b=4),
                            in_=dT_r[:, 16 + 4 * j:16 + 4 * j + 4])
                        rb_lo = W.tile([128, SQC], F32, tag="rblo",
                                       name="rb_lo")
                        rb_hi = W.tile([128, SQC], F32, tag="rbhi",
                                       name="rb_hi")
                        nc.gpsimd.partition_broadcast(
                            rb_lo, rrow_lo[0:1, SQC * j:SQC * (j + 1)])
                        nc.vector.tensor_tensor(
                            attn[j][0:64, SQC * c:SQC * (c + 1)],
                            asl[0:64, :], rb_lo[0:64, :], MUL)
                        nc.gpsimd.partition_broadcast(
                            rb_hi, rrow_hi[0:1, SQC * j:SQC * (j + 1)])
                        nc.vector.tensor_tensor(
                            attn[j][64:128, SQC * c:SQC * (c + 1)],
                            ash[64:128, :], rb_hi[64:128, :], MUL)

                    fl_cnt = 0
                    for j in range(NPAIR):
                        pp = j % 2
                        av_lo = AVPS.tile([128, SQC], F32, tag=f"avlo{pp}",
                                          name="av_lo")
                        av_hi = AVPS.tile([128, SQC], F32, tag=f"avhi{pp}",
                                          name="av_hi")
                        # software pipeline: QK/exp for tile ti runs two steps
                        # ahead of the AV matmuls, so AV never waits on exp
                        pend_av = []

                        def emit_av(ent):
                            t, r, first, last, p = ent
                            nc.tensor.matmul(
                                av_lo[0:96, r:SQC],
                                v0_sb[:, V0W * t:V0W * t + V0W],
                                p[:, r:SQC],
                                start=first, stop=last,
                            )
                            nc.tensor.matmul(
                                av_hi[0:128, r:SQC],
                                v1_sb[:, V1W * t:V1W * t + V1W],
                                p[:, SQC + r:2 * SQC],
                                start=first, stop=last,
                            )

                        def pop_fillers():
                            nonlocal fill_acc, fl_cnt
                            if j == 0:
                                return
                            fill_acc += fill_rate
                            while fill_acc >= 1.0 and fillers:
                                bank = "avlo" if fl_cnt % 2 == 0 else "avhi"
                                fillers.pop(0)(f"{bank}{(j + 1) % 2}")
                                fl_cnt += 1
                                fill_acc -= 1.0

                        for ti, (t, diag, r0) in enumerate(glist):
                            first = ti == 0
                            last = ti == len(glist) - 1
                            r = 0 if first else r0
                            sc = SCPS.tile([128, 2 * SQC], F32,
                                           tag="sc", name="sc")
                            mm_stop = (not diag) or MASK_ON_DVE
                            nc.tensor.matmul(
                                sc[:, r:SQC],
                                k_t[0:64, SKT * t:SKT * (t + 1)],
                                q_t[j][0:64, SQC * c + r:SQC * (c + 1)],
                                start=True, stop=mm_stop,
                            )
                            nc.tensor.matmul(
                                sc[:, SQC + r:2 * SQC],
                                k_t[64:128, SKT * t:SKT * (t + 1)],
                                q_t[j][64:128, SQC * c + r:SQC * (c + 1)],
                                start=True, stop=mm_stop,
                            )
                            if diag and not MASK_ON_DVE:
                                nc.tensor.matmul(
                                    sc[:, r0:r0 + 128], ident, dm_sb,
                                    start=False, stop=True,
                                )
                                nc.tensor.matmul(
                                    sc[:, SQC + r0:SQC + r0 + 128], ident,
                                    dm_sb, start=False, stop=True,
                                )
                            p = W.tile([128, 2 * SQC], BF, tag="p", name="p",
                                       bufs=4)
                            sc3 = sc.rearrange("q (h f) -> q h f", h=2)
                            p3 = p.rearrange("q (h f) -> q h f", h=2)
                            if not (diag and MASK_ON_DVE):
                                if r:
                                    nc.scalar.activation(p3[:, :, r:SQC],
                                                         sc3[:, :, r:SQC],
                                                         EXP, scale=0.125)
                                else:
                                    nc.scalar.activation(p, sc, EXP,
                                                         scale=0.125)
                            else:
                                # diag tile: exp the fully-valid columns into
                                # p directly; exp the 128-wide diagonal block
                                # into a temp, then 0/1-triangle multiply it
                                # into p on DVE (single writer per p region)
                                if r0 + 128 < SQC:
                                    nc.scalar.activation(
                                        p3[:, :, r0 + 128:SQC],
                                        sc3[:, :, r0 + 128:SQC],
                                        EXP, scale=0.125)
                                tt = W.tile([128, 256], BF, tag="texp",
                                            name="texp", bufs=3)
                                tt3 = tt.rearrange("q (h f) -> q h f", h=2)
                                nc.scalar.activation(
                                    tt3, sc3[:, :, r0:r0 + 128],
                                    EXP, scale=0.125)
                                nc.vector.tensor_tensor(
                                    p[:, r0:r0 + 128], tt[:, 0:128],
                                    dm_sb, MUL)
                                nc.vector.tensor_tensor(
                                    p[:, SQC + r0:SQC + r0 + 128],
                                    tt[:, 128:256], dm_sb, MUL)
                            if len(pend_av) == 2:
                                emit_av(pend_av.pop(0))
                                pop_fillers()
                            pend_av.append((t, r, first, last, p))
                        for ent in pend_av:
                            emit_av(ent)
                            pop_fillers()
                        # evacuate av psum to sbuf (frees the bank set) and
                        # scatter this pair's denominator row into dT
                        asl = AVS.tile([128, SQC], F32, tag="avsblo",
                                       name="av_sb_lo", bufs=4)
                        ash = AVS.tile([128, SQC], F32, tag="avsbhi",
                                       name="av_sb_hi", bufs=4)
                        nc.vector.tensor_copy(asl[0:96, :], av_lo[0:96, :])
                        nc.vector.tensor_copy(ash[64:128, :],
                                              av_hi[64:128, :])
                        nc.vector.tensor_copy(ash[32:33, :],
                                              av_hi[32:33, :])
                        nc.sync.dma_start(out=dT[:, 4 * j:4 * j + 4],
                                          in_=asl[64:65, :])
                        nc.sync.dma_start(out=dT[:, 16 + 4 * j:16 + 4 * j + 4],
                                          in_=ash[32:33, :])
                        pair_norm(j, asl, ash)
                # drain remaining wo work (chunk 3)
                fillers.extend(wo_chunk_thunks(NCHUNK - 1))
                tags = ["avlo0", "avhi0", "avlo1", "avhi1"]
                for i, f in enumerate(fillers):
                    f(tags[i % 4])

    nc.finalize()
    return nc


def kernel(**inputs):
    global last_exec_time_ns, last_trace
    from concourse.bass_utils import run_bass_kernel_spmd

    x = np.asarray(inputs["x"], np.float32)
    freqs_cos = np.asarray(inputs["freqs_cos"], np.float32)
    freqs_sin = np.asarray(inputs["freqs_sin"], np.float32)
    mask = np.asarray(inputs["mask"], np.float32)
    wq = np.asarray(inputs["wq"], np.float32)
    wk = np.asarray(inputs["wk"], np.float32)
    wv = np.asarray(inputs["wv"], np.float32)
    wo = np.asarray(inputs["wo"], np.float32)

    chunks = _mask_structure(mask)
    # diagonal-block mask: 0/1 triangle (DVE) or additive -240 (PE)
    tri = np.arange(128)[None, :] >= np.arange(128)[:, None]
    dmask = (np.where(tri, 1.0, 0.0) if MASK_ON_DVE
             else np.where(tri, 0.0, -240.0)).astype(BF16)

    if chunks not in _build_cache:
        _build_cache[chunks] = _build(chunks)
    nc = _build_cache[chunks]

    # trig tiles in pair layout (same for both heads of a pair)
    fi2 = np.tile(_freq, 2)
    sg2 = np.tile(_sgn, 2)
    c1 = freqs_cos.T[fi2].astype(BF16)                      # [128, S]
    c2 = (freqs_sin.T[fi2] * sg2[:, None]).astype(BF16)     # [128, S]

    # pair j holds (q-head j, q-head j+4) so lo half uses kv 0, hi half kv 1
    pair_order = [0, 4, 1, 5, 2, 6, 3, 7]
    q_cols = np.concatenate([64 * pair_order[i] + _perm
                             for i in range(H // TP)])
    o_rows = np.concatenate([np.arange(64 * pair_order[i],
                                       64 * pair_order[i] + 64)
                             for i in range(H // TP)])
    kv_perm = np.concatenate([64 * h + _perm for h in range(KV // TP)])

    in_maps = []
    for d in range(DP):
        xt = np.ascontiguousarray(x[d].T).astype(BF16)
        for t in range(TP):
            wq_s = np.ascontiguousarray(
                wq[:, HQ * t:HQ * (t + 1)][:, q_cols]).astype(BF16)
            wk_s = np.ascontiguousarray(
                wk[:, HKV * t:HKV * (t + 1)][:, kv_perm]).astype(BF16)
            wv_s = np.ascontiguousarray(
                wv[:, HKV * t:HKV * (t + 1)]).astype(BF16)
            wo_s = np.ascontiguousarray(
                wo[HQ * t:HQ * (t + 1), :][o_rows]).astype(BF16)
            in_maps.append({
                "xt": xt, "wq": wq_s, "wk": wk_s, "wv": wv_s, "wo": wo_s,
                "c1": c1, "c2": c2, "dmask": dmask,
            })

    trace = bool(os.environ.get("BASS_KERNEL_TRACE"))
    res = run_bass_kernel_spmd(nc, in_maps, core_ids=list(range(NCORES)),
                               trace=trace)
    last_exec_time_ns = res.exec_time_ns
    last_trace = res
    out = np.empty((B, S, DIM), np.float32)
    for d in range(DP):
        acc = res.results[d * TP]["out"].astype(np.float32)
        for t in range(1, TP):
            acc = acc + res.results[d * TP + t]["out"]
        out[d] = acc
    return out
